# revision 1
# baseline (speedup 1.0000x reference)
"""GATConv forward on 8 Trainium2 NeuronCores (Bass/Tile).

Strategy: destination-node sharding. Host sorts edges by dst, assigns each
core a contiguous dst range (12544 nodes = 98 tiles of 128). Node ids are
cyclically renumbered per core so every core's local nodes are 0..12543 and
the SPMD program is identical across cores; all per-core variation lives in
the input data.

Device phase A: h_aug = x @ [W | W@att_src_h | W@att_dst_h] (bf16 matmul via
DMA-transpose of x), written as a [N_pad, 256]-bf16 row table T in HBM
(cols 0:128 = h, 128:132 = a_src, 132:136 = a_dst); a_dst for the core's
local tiles is kept in SBUF.

Device phase B, per output tile (128 dst nodes): dma_gather T rows by edge
src (4 src banks of 32768 rows for int16 indices), build a one-hot
slot-vs-iota matrix S01 per 128-edge chunk, expand a_dst to edges with a
PE transpose + small matmul, w = exp(leakyrelu(a_src+a_dst)), scatter
Σ w·h and Σ w into PSUM via S01^T @ [w*h | w], then out = relu(mean_h
(Σ w·h)/(Σ w) + bias).
"""
import sys

sys.path.insert(0, "/opt/trn_rl_repo")
import numpy as np
import ml_dtypes

import concourse.bass as bass
import concourse.mybir as mybir
import concourse.tile as tile
from concourse.bass_utils import run_bass_kernel_spmd
from concourse import bacc

BF16 = ml_dtypes.bfloat16
P = 128
N = 100000
NPAD = 100352          # 784 tiles of 128; 8 cores x 12544
NCORE = 8
B = NPAD // NCORE      # 12544 local nodes per core
TPC = B // P           # 98 tiles per core
NTILE = NPAD // P      # 784 global row tiles
BANK = 32768
NBANK = (NPAD + BANK - 1) // BANK  # 4
EL = 256               # table row: 256 bf16 cols = 512B
NEG = 0.2
H, C = 4, 32
PAD_SLOT = 200.0


def _prep_edges(edge_index):
    src0 = edge_index[0].astype(np.int64)
    dst0 = edge_index[1].astype(np.int64)
    loops = np.arange(NPAD, dtype=np.int64)
    src = np.concatenate([src0, loops])
    dst = np.concatenate([dst0, loops])

    per_core = []
    cnts = np.zeros((NCORE, TPC * NBANK), np.int64)
    for c in range(NCORE):
        lo, hi = c * B, (c + 1) * B
        sel = (dst >= lo) & (dst < hi)
        d = dst[sel] - lo
        s = (src[sel] - lo) % NPAD
        t = d >> 7
        sl = d & 127
        bk = s >> 15
        il = s & (BANK - 1)
        g = t * NBANK + bk
        order = np.argsort(g, kind="stable")
        gs = g[order]
        per_core.append((gs, il[order], sl[order]))
        cnts[c] = np.bincount(g, minlength=TPC * NBANK)

    K = np.ceil(cnts.max(axis=0) / P).astype(np.int64)  # chunks per (t,b), shared
    sz = K * P
    goff = np.zeros(TPC * NBANK + 1, np.int64)
    np.cumsum(sz, out=goff[1:])
    tot = int(goff[-1])

    idx_maps, slot_maps, dst_maps = [], [], []
    for c in range(NCORE):
        gs, ils, sls = per_core[c]
        cnt = cnts[c]
        start = np.zeros(TPC * NBANK + 1, np.int64)
        np.cumsum(cnt, out=start[1:])
        rank = np.arange(len(gs)) - start[gs]
        pos = goff[gs] + rank
        idx_pad = np.zeros(tot, np.int16)
        idx_pad[pos] = ils.astype(np.int16)
        slot_pad = np.full(tot, PAD_SLOT, np.float32)
        slot_pad[pos] = sls.astype(np.float32)
        dl_pad = np.zeros(tot, np.int16)
        dl_pad[pos] = ((gs // NBANK) * P + sls).astype(np.int16)

        iblocks, sblocks, dblocks = [], [], []
        for g in range(TPC * NBANK):
            n_g = int(sz[g])
            if n_g == 0:
                continue
            seg = idx_pad[goff[g]:goff[g] + n_g]
            iblocks.append(np.ascontiguousarray(seg.reshape(n_g // 16, 16).T))
            seg2 = slot_pad[goff[g]:goff[g] + n_g]
            sblocks.append(np.ascontiguousarray(seg2.reshape(n_g // P, P).T))
            seg3 = dl_pad[goff[g]:goff[g] + n_g]
            dblocks.append(np.ascontiguousarray(seg3.reshape(n_g // 16, 16).T))
        idx16 = np.concatenate(iblocks, axis=1)          # [16, tot/16]
        idx128 = np.ascontiguousarray(np.tile(idx16, (8, 1)))
        slotf = np.concatenate(sblocks, axis=1).astype(BF16)  # [128, nchunks]
        dst16 = np.concatenate(dblocks, axis=1)
        dst128 = np.ascontiguousarray(np.tile(dst16, (8, 1)))
        idx_maps.append(idx128)
        slot_maps.append(np.ascontiguousarray(slotf))
        dst_maps.append(dst128)
    return K.reshape(TPC, NBANK), idx_maps, slot_maps, dst_maps


def _build_program(K):
    TOTC = int(K.sum())           # total chunks
    TOT16 = TOTC * 8              # idx cols (int16) per partition row
    f32, bf16, i16 = mybir.dt.float32, mybir.dt.bfloat16, mybir.dt.int16
    AF = mybir.ActivationFunctionType
    OP = mybir.AluOpType

    nc = bacc.Bacc("TRN2", target_bir_lowering=False, debug=False,
                   num_devices=NCORE)
    xbf = nc.dram_tensor("xbf", [NPAD, P], bf16, kind="ExternalInput")
    waug = nc.dram_tensor("waug", [P, 136], bf16, kind="ExternalInput")
    idx_all = nc.dram_tensor("idx_all", [P, TOT16], i16, kind="ExternalInput")
    slot_all = nc.dram_tensor("slot_all", [P, TOTC], bf16, kind="ExternalInput")
    dst_all = nc.dram_tensor("dst_all", [P, TOT16], i16, kind="ExternalInput")
    iota_in = nc.dram_tensor("iota_in", [P, P], bf16, kind="ExternalInput")
    ident_in = nc.dram_tensor("ident_in", [P, P], bf16, kind="ExternalInput")
    bias_in = nc.dram_tensor("bias_in", [P, C], f32, kind="ExternalInput")
    T = nc.dram_tensor("T", [NPAD, EL], bf16)
    A = nc.dram_tensor("A", [B, 128], bf16)
    out_d = nc.dram_tensor("out", [B, C], f32, kind="ExternalOutput")

    Tv = T[:, :].rearrange("(t p) e -> p t e", p=P)       # [128, 784, 256]
    Av = A[:, :].rearrange("(t p) e -> p t e", p=P)       # [128, 98, 128]
    out_v = out_d[:, :].rearrange("(t p) c -> p t c", p=P)  # [128, 98, 32]

    with tile.TileContext(nc) as tc:
        with tc.tile_pool(name="const", bufs=1) as cp:
            waug_sb = cp.tile([P, 136], bf16)
            nc.sync.dma_start(out=waug_sb[:], in_=waug[:, :])
            iota_sb = cp.tile([P, P], bf16)
            nc.sync.dma_start(out=iota_sb[:], in_=iota_in[:, :])
            ident_sb = cp.tile([P, P], bf16)
            nc.sync.dma_start(out=ident_sb[:], in_=ident_in[:, :])
            bias_sb = cp.tile([P, C], f32)
            nc.sync.dma_start(out=bias_sb[:], in_=bias_in[:, :])
            idx_sb = cp.tile([P, TOT16], i16)
            nc.sync.dma_start(out=idx_sb[:], in_=idx_all[:, :])
            slot_sb = cp.tile([P, TOTC], bf16)
            nc.sync.dma_start(out=slot_sb[:], in_=slot_all[:, :])
            dst_sb = cp.tile([P, TOT16], i16)
            nc.sync.dma_start(out=dst_sb[:], in_=dst_all[:, :])
            adst_sb = cp.tile([P, TPC, 4], bf16)
            outall_sb = cp.tile([P, TPC, C], f32)

            # ---------------- Phase A: build T = [h | a_src | a_dst] -------
            with tc.tile_pool(name="pa", bufs=3) as pa, \
                 tc.tile_pool(name="psa", bufs=4, space="PSUM") as psa:
                for rb in range(NPAD // 512):
                    xT = pa.tile([P, 512], bf16, tag="xT")
                    nc.sync.dma_start(out=xT[:], in_=xbf[rb * 512:(rb + 1) * 512, :],
                                      transpose=True)
                    Tb = pa.tile([P, 4, EL], bf16, tag="Tb")
                    for i in range(4):
                        tg = rb * 4 + i
                        ps = psa.tile([P, 136], f32, tag="psA", space="PSUM")
                        nc.tensor.matmul(out=ps[:], lhsT=xT[:, i * P:(i + 1) * P],
                                         rhs=waug_sb[:], start=True, stop=True)
                        nc.scalar.activation(out=Tb[:, i, 0:136], in_=ps[:, 0:136],
                                             func=AF.Copy)
                        if tg < TPC:
                            nc.vector.tensor_copy(out=adst_sb[:, tg, :],
                                                  in_=ps[:, 132:136])
                    nc.sync.dma_start(out=Tv[:, rb * 4:(rb + 1) * 4, :], in_=Tb[:])

            nc.sync.dma_start(out=Av[:, :, 0:4], in_=adst_sb[:])
            tc.strict_bb_all_engine_barrier()

            # ---------------- Phase B: gather / scatter --------------------
            with tc.tile_pool(name="pb", bufs=3) as pb, \
                 tc.tile_pool(name="psb", bufs=2, space="PSUM") as psb:
                cc = 0
                for t in range(TPC):
                    accp = psb.tile([P, 132], f32, tag="acc", space="PSUM")
                    nch = int(K[t].sum())
                    done = 0
                    for b in range(NBANK):
                        Kb = int(K[t, b])
                        if Kb == 0:
                            continue
                        rows = min(BANK, NPAD - b * BANK)
                        gt = pb.tile([P, Kb, EL], bf16, tag="gath")
                        nc.gpsimd.dma_gather(
                            out_ap=gt[:],
                            in_ap=T[b * BANK:b * BANK + rows, :],
                            idxs_ap=idx_sb[:, cc * 8:(cc + Kb) * 8],
                            num_idxs=Kb * P, num_idxs_reg=Kb * P, elem_size=EL)
                        s01g = pb.tile([P, Kb, P], bf16, tag="s01")
                        nc.vector.tensor_tensor(
                            out=s01g[:],
                            in0=slot_sb[:, cc:cc + Kb, None].to_broadcast([P, Kb, P]),
                            in1=iota_sb[:, None, :].to_broadcast([P, Kb, P]),
                            op=OP.is_equal)
                        adt = pb.tile([P, Kb, 128], bf16, tag="adt")
                        nc.gpsimd.dma_gather(
                            out_ap=adt[:], in_ap=A[:, :],
                            idxs_ap=dst_sb[:, cc * 8:(cc + Kb) * 8],
                            num_idxs=Kb * P, num_idxs_reg=Kb * P, elem_size=128)
                        zt = pb.tile([P, Kb, 4], f32, tag="zt")
                        nc.vector.tensor_tensor(
                            out=zt[:], in0=gt[:, :, 128:132],
                            in1=adt[:, :, 0:4],
                            op=OP.add)
                        lr = pb.tile([P, Kb * 4], f32, tag="lr")
                        nc.vector.scalar_tensor_tensor(
                            out=lr[:], in0=zt[:].rearrange("p k f -> p (k f)"),
                            scalar=NEG, in1=zt[:].rearrange("p k f -> p (k f)"),
                            op0=OP.mult, op1=OP.max)
                        wb = pb.tile([P, Kb * 4], bf16, tag="wb")
                        nc.scalar.activation(out=wb[:], in_=lr[:], func=AF.Exp)
                        msg = pb.tile([P, Kb, 132], bf16, tag="msg")
                        nc.vector.tensor_tensor(
                            out=msg[:, :, 0:128].rearrange("p k (h c) -> p k h c", h=H),
                            in0=gt[:, :, 0:128].rearrange("p k (h c) -> p k h c", h=H),
                            in1=wb[:].rearrange("p (k h) -> p k h", h=H)[:, :, :, None]
                                .to_broadcast([P, Kb, H, C]),
                            op=OP.mult)
                        nc.vector.tensor_copy(
                            out=msg[:, :, 128:132],
                            in_=wb[:].rearrange("p (k f) -> p k f", f=4))
                        for k in range(Kb):
                            nc.tensor.matmul(out=accp[:], lhsT=s01g[:, k, :],
                                             rhs=msg[:, k, :],
                                             start=(done == 0), stop=(done == nch - 1))
                            done += 1
                        cc += Kb
                    # epilogue
                    rec = pb.tile([P, 4], f32, tag="rec")
                    nc.vector.reciprocal(out=rec[:], in_=accp[:, 128:132])
                    rec2 = pb.tile([P, 4], f32, tag="rec2")
                    nc.vector.tensor_scalar_mul(out=rec2[:], in0=rec[:], scalar1=1.0 / H)
                    tmp = pb.tile([P, P], f32, tag="tmp")
                    nc.vector.tensor_tensor(
                        out=tmp[:].rearrange("p (h c) -> p h c", h=H),
                        in0=accp[:, 0:128].rearrange("p (h c) -> p h c", h=H),
                        in1=rec2[:, :, None].to_broadcast([P, H, C]),
                        op=OP.mult)
                    hsum = pb.tile([P, C], f32, tag="hsum")
                    nc.vector.tensor_reduce(
                        out=hsum[:], in_=tmp[:].rearrange("p (h c) -> p c h", h=H),
                        axis=mybir.AxisListType.X, op=OP.add)
                    badd = pb.tile([P, C], f32, tag="badd")
                    nc.vector.tensor_add(out=badd[:], in0=hsum[:], in1=bias_sb[:])
                    nc.vector.tensor_scalar_max(out=outall_sb[:, t, :], in0=badd[:],
                                                scalar1=0.0)
                nc.sync.dma_start(out=out_v[:, :, :], in_=outall_sb[:])
    nc.compile()
    return nc


def prepare(x, edge_index, W, att_src, att_dst, bias):
    x = np.asarray(x, np.float32)
    W = np.asarray(W, np.float32)
    att_src = np.asarray(att_src, np.float32)
    att_dst = np.asarray(att_dst, np.float32)
    bias = np.asarray(bias, np.float32)

    wa = np.zeros((P, 136), np.float32)
    wa[:, :128] = W
    for hh in range(H):
        wa[:, 128 + hh] = W[:, hh * C:(hh + 1) * C] @ att_src[hh]
        wa[:, 132 + hh] = W[:, hh * C:(hh + 1) * C] @ att_dst[hh]
    wa_bf = wa.astype(BF16)

    x_pad = np.zeros((NPAD, P), np.float32)
    x_pad[:N] = x
    x_bf = x_pad.astype(BF16)

    K, idx_maps, slot_maps, dst_maps = _prep_edges(np.asarray(edge_index))
    nc = _build_program(K)

    iota_np = np.tile(np.arange(P, dtype=np.float32)[None, :], (P, 1)).astype(BF16)
    ident_np = np.eye(P, dtype=np.float32).astype(BF16)
    bias_rep = np.tile(bias[None, :], (P, 1)).astype(np.float32)

    in_maps = []
    for c in range(NCORE):
        xc = np.roll(x_bf, -c * B, axis=0)
        in_maps.append({
            "xbf": np.ascontiguousarray(xc),
            "waug": wa_bf,
            "idx_all": idx_maps[c],
            "slot_all": slot_maps[c],
            "dst_all": dst_maps[c],
            "iota_in": iota_np,
            "ident_in": ident_np,
            "bias_in": bias_rep,
        })
    return nc, in_maps


def kernel(x, edge_index, W, att_src, att_dst, bias):
    nc, in_maps = prepare(x, edge_index, W, att_src, att_dst, bias)
    res = run_bass_kernel_spmd(nc, in_maps, list(range(NCORE)))
    out = np.empty((NPAD, C), np.float32)
    for c in range(NCORE):
        out[c * B:(c + 1) * B] = res.results[c]["out"]
    return out[:N]



# revision 9
# speedup vs baseline: 1.0534x; 1.0534x over previous
"""GATConv forward on 8 Trainium2 NeuronCores (Bass/Tile).

Strategy: destination-node sharding. Host sorts edges by dst, assigns each
core a contiguous dst range (12544 nodes = 98 tiles of 128). Node ids are
cyclically renumbered per core so every core's local nodes are 0..12543 and
the SPMD program is identical across cores; all per-core variation lives in
the input data.

Phase A: T = x @ W (bf16, 256B rows) written tile-by-tile to HBM;
a_dst per local node kept in SBUF (slot-partitioned, bf16).

Phase B: dst tiles are processed in groups of 4 (one dma_gather per
(group, src-bank) to amortize the ~1us fixed SWDGE cost per gather).
Per-edge a_src = <h_src, att_src> is computed on DVE from the gathered
rows. Per-edge a_dst is expanded from the SBUF-resident per-tile table
via tiny PE matmuls against a transposed one-hot S01T, which is built by
bit-expanding a host-packed uint16 bitmap (no second gather). Messages
w*h and w are scattered into per-tile PSUM accumulators with one-hot
matmuls, then out = relu(mean_h (sum w*h)/(sum w) + bias).
"""
import sys

sys.path.insert(0, "/opt/trn_rl_repo")
import numpy as np
import ml_dtypes

import concourse.bass as bass
import concourse.mybir as mybir
import concourse.tile as tile
from concourse.bass_utils import run_bass_kernel_spmd
from concourse import bacc

BF16 = ml_dtypes.bfloat16
P = 128
N = 100000
NPAD = 100352          # 784 tiles of 128; 8 cores x 12544
NCORE = 8
B = NPAD // NCORE      # 12544 local nodes per core
TPC = B // P           # 98 tiles per core
NTILE = NPAD // P      # 784 global row tiles
BANK = 32768
NBANK = (NPAD + BANK - 1) // BANK  # 4
NEG = 0.2
H, C = 4, 32
PAD_SLOT = 200.0
G = 4                  # dst tiles per gather group
GROUPS = [list(range(g, min(g + G, TPC))) for g in range(0, TPC, G)]


def _prep_edges(edge_index):
    src0 = edge_index[0].astype(np.int64)
    dst0 = edge_index[1].astype(np.int64)
    loops = np.arange(NPAD, dtype=np.int64)
    src = np.concatenate([src0, loops])
    dst = np.concatenate([dst0, loops])

    per_core = []
    cnts = np.zeros((NCORE, TPC * NBANK), np.int64)
    for c in range(NCORE):
        lo, hi = c * B, (c + 1) * B
        sel = (dst >= lo) & (dst < hi)
        d = dst[sel] - lo
        s = (src[sel] - lo) % NPAD
        t = d >> 7
        sl = d & 127
        bk = s >> 15
        il = s & (BANK - 1)
        q = t * NBANK + bk
        per_core.append((q, il, sl))
        cnts[c] = np.bincount(q, minlength=TPC * NBANK)

    K = np.ceil(cnts.max(axis=0) / P).astype(np.int64).reshape(TPC, NBANK)

    # global chunk layout: (group, bank, tile, chunk)
    # qorder[pos] = q id at that position
    qorder = []
    for tiles in GROUPS:
        for bk in range(NBANK):
            for t in tiles:
                qorder.append(t * NBANK + bk)
    qorder = np.array(qorder, np.int64)
    sz_by_q = (K.reshape(-1) * P)           # padded edges per q
    sz_in_order = sz_by_q[qorder]
    goff_in_order = np.zeros(len(qorder) + 1, np.int64)
    np.cumsum(sz_in_order, out=goff_in_order[1:])
    tot = int(goff_in_order[-1])            # total padded edges
    # offset of each q id in the global edge order
    qoff = np.zeros(TPC * NBANK, np.int64)
    qoff[qorder] = goff_in_order[:-1]

    TOTC = tot // P                         # total chunks
    idx_maps, slot_maps, bm_maps = [], [], []
    for c in range(NCORE):
        q, il, sl = per_core[c]
        cnt = cnts[c]
        start = np.zeros(TPC * NBANK + 1, np.int64)
        np.cumsum(cnt, out=start[1:])
        order = np.argsort(q, kind="stable")
        qs = q[order]
        rank = np.arange(len(qs)) - start[qs]
        pos = qoff[qs] + rank               # global edge position
        idx_pad = np.zeros(tot, np.int16)
        idx_pad[pos] = il[order].astype(np.int16)
        slot_pad = np.full(tot, PAD_SLOT, np.float32)
        slot_pad[pos] = sl[order].astype(np.float32)

        # idx table: per chunk [16, 8] wrap -> [16, tot/16], replicated to 128
        idx16 = np.ascontiguousarray(
            idx_pad.reshape(TOTC, 8, 16).transpose(2, 0, 1).reshape(16, TOTC * 8))
        idx128 = np.ascontiguousarray(np.tile(idx16, (8, 1)))
        # slot table: [128, TOTC] edge-partitioned
        slotT = np.ascontiguousarray(
            slot_pad.reshape(TOTC, P).T.astype(BF16))
        # bitmap: bm[s, jj] bit b set iff slot of edge jj*8+b == s
        bm = np.zeros((P, TOTC * 16), np.uint8)
        j = np.nonzero(slot_pad < P)[0]
        rows = slot_pad[j].astype(np.int64)
        np.bitwise_or.at(bm, (rows, j >> 3), (1 << (j & 7)).astype(np.uint8))
        idx_maps.append(idx128)
        slot_maps.append(slotT)
        bm_maps.append(np.ascontiguousarray(bm))
    return K, idx_maps, slot_maps, bm_maps


def _plan(K):
    """Per-group program metadata (shared across cores)."""
    plan = []
    cc = 0
    for tiles in GROUPS:
        gathers = []   # (bank, nch, chunk_off_in_group); nch <= 8 (1024-desc
                       # SWDGE ring limit per call on real hw)
        chunks = {}    # (bk, t) -> (local chunk offset, nch)
        off = 0
        for bk in range(NBANK):
            nch_bk = int(K[tiles, bk].sum())
            for p0 in range(0, nch_bk, 8):
                gathers.append((bk, min(8, nch_bk - p0), off + p0))
            o = off
            for t in tiles:
                chunks[(bk, t)] = (o, int(K[t, bk]))
                o += int(K[t, bk])
            off += nch_bk
        kg = off
        # scatter matmul order: tile-major so one accumulation group is
        # open per accp at a time
        scatter = []   # (t_local, local chunk idx, start, stop)
        for ti, t in enumerate(tiles):
            nt = int(K[t].sum())
            done = 0
            for bk in range(NBANK):
                o, n = chunks[(bk, t)]
                for k in range(n):
                    scatter.append((ti, o + k, done == 0, done == nt - 1))
                    done += 1
        plan.append(dict(tiles=tiles, gathers=gathers, kg=kg, cc0=cc,
                         scatter=scatter))
        cc += kg
    return plan, cc


def _build_program(K):
    plan, TOTC = _plan(K)
    TOT16 = TOTC * 8
    f32, bf16, i16, u8 = (mybir.dt.float32, mybir.dt.bfloat16,
                          mybir.dt.int16, mybir.dt.uint8)
    AF = mybir.ActivationFunctionType
    OP = mybir.AluOpType

    nc = bacc.Bacc("TRN2", target_bir_lowering=False, debug=False,
                   num_devices=NCORE)
    xbf = nc.dram_tensor("xbf", [NPAD, P], bf16, kind="ExternalInput")
    waug = nc.dram_tensor("waug", [P, 132], bf16, kind="ExternalInput")
    idx_all = nc.dram_tensor("idx_all", [P, TOT16], i16, kind="ExternalInput")
    slot_all = nc.dram_tensor("slot_all", [P, TOTC], bf16, kind="ExternalInput")
    bm_all = nc.dram_tensor("bm_all", [P, TOTC * 16], u8, kind="ExternalInput")
    iota_in = nc.dram_tensor("iota_in", [P, P], bf16, kind="ExternalInput")
    atts_in = nc.dram_tensor("atts_in", [P, P], bf16, kind="ExternalInput")
    bias_in = nc.dram_tensor("bias_in", [P, C], f32, kind="ExternalInput")
    T = nc.dram_tensor("T", [NPAD, P], bf16)
    out_d = nc.dram_tensor("out", [B, C], f32, kind="ExternalOutput")

    Tv = T[:, :].rearrange("(t p) e -> p t e", p=P)         # [128, 784, 128]
    out_v = out_d[:, :].rearrange("(t p) c -> p t c", p=P)  # [128, 98, 32]

    with tile.TileContext(nc) as tc:
        with tc.tile_pool(name="const", bufs=1) as cp:
            waug_sb = cp.tile([P, 132], bf16)
            nc.sync.dma_start(out=waug_sb[:], in_=waug[:, :])
            iota_sb = cp.tile([P, P], bf16)
            nc.sync.dma_start(out=iota_sb[:], in_=iota_in[:, :])
            atts_sb = cp.tile([P, P], bf16)
            nc.sync.dma_start(out=atts_sb[:], in_=atts_in[:, :])
            bias_sb = cp.tile([P, C], f32)
            nc.sync.dma_start(out=bias_sb[:], in_=bias_in[:, :])
            idx_sb = cp.tile([P, TOT16], i16)
            nc.sync.dma_start(out=idx_sb[:], in_=idx_all[:, :])
            slot_sb = cp.tile([P, TOTC], bf16)
            nc.sync.dma_start(out=slot_sb[:], in_=slot_all[:, :])
            adst_sb = cp.tile([P, TPC, 4], bf16)
            outall_sb = cp.tile([P, TPC, C], f32)

            # ---------------- Phase A: T = x @ W, a_dst table ---------------
            NB = NPAD // 1024  # 98 blocks of 1024 rows (8 tiles)
            with tc.tile_pool(name="pa", bufs=3) as pa, \
                 tc.tile_pool(name="psa", bufs=2, space="PSUM") as psa:
                for rb in range(NB):
                    xT = pa.tile([P, 1024], bf16, tag="xT")
                    nc.sync.dma_start(out=xT[:],
                                      in_=xbf[rb * 1024:(rb + 1) * 1024, :],
                                      transpose=True)
                    Tb = pa.tile([P, 8, P], bf16, tag="Tb")
                    for half in range(2):
                        ps = psa.tile([P, 4, 512], f32, tag="psA", space="PSUM")
                        for i in range(4):
                            nc.tensor.matmul(
                                out=ps[:, i, 0:132],
                                lhsT=xT[:, (half * 4 + i) * P:(half * 4 + i + 1) * P],
                                rhs=waug_sb[:], start=True, stop=True)
                        nc.vector.tensor_copy(
                            out=Tb[:, half * 4:(half + 1) * 4, :],
                            in_=ps[:, :, 0:128])
                        t0 = rb * 8 + half * 4
                        if t0 < TPC:
                            nloc = min(4, TPC - t0)
                            nc.vector.tensor_copy(
                                out=adst_sb[:, t0:t0 + nloc, :],
                                in_=ps[:, 0:nloc, 128:132])
                    nc.scalar.dma_start(out=Tv[:, rb * 8:(rb + 1) * 8, :],
                                        in_=Tb[:])

            tc.strict_bb_all_engine_barrier()

            # ---------------- Phase B: gather / scatter --------------------
            with tc.tile_pool(name="pgt", bufs=2) as pgt, \
                 tc.tile_pool(name="pbm", bufs=2) as pbm, \
                 tc.tile_pool(name="ptr", bufs=1) as ptr, \
                 tc.tile_pool(name="psm", bufs=2) as psm, \
                 tc.tile_pool(name="psb", bufs=5, space="PSUM") as psb, \
                 tc.tile_pool(name="psd", bufs=2, space="PSUM") as psd:
                for g, pl in enumerate(plan):
                    kg, cc0, tiles = pl["kg"], pl["cc0"], pl["tiles"]
                    gt = pgt.tile([P, kg, P], bf16, tag="gath")
                    for bk, nch, off in pl["gathers"]:
                        rows = min(BANK, NPAD - bk * BANK)
                        nc.gpsimd.dma_gather(
                            out_ap=gt[:, off:off + nch, :],
                            in_ap=T[bk * BANK:bk * BANK + rows, :],
                            idxs_ap=idx_sb[:, (cc0 + off) * 8:(cc0 + off + nch) * 8],
                            num_idxs=nch * P, num_idxs_reg=nch * P,
                            elem_size=P)
                    bm_t = pbm.tile([P, kg * 16], u8, tag="bm")
                    nc.scalar.dma_start(out=bm_t[:],
                                        in_=bm_all[:, cc0 * 16:(cc0 + kg) * 16])
                    # edge-partitioned one-hot (for scatter)
                    s01g = ptr.tile([P, kg, P], bf16, tag="s01")
                    nc.vector.tensor_tensor(
                        out=s01g[:],
                        in0=slot_sb[:, cc0:cc0 + kg, None].to_broadcast([P, kg, P]),
                        in1=iota_sb[:, None, :].to_broadcast([P, kg, P]),
                        op=OP.is_equal)
                    # slot-partitioned one-hot (for a_dst expansion)
                    bx = ptr.tile([P, kg * 16, 8], u8, tag="bx")
                    for b in range(8):
                        nc.vector.tensor_scalar(
                            out=bx[:, :, b], in0=bm_t[:],
                            scalar1=1 << b, scalar2=b,
                            op0=OP.bitwise_and, op1=OP.logical_shift_right)
                    s01T = ptr.tile([P, kg * P], bf16, tag="s01T")
                    nc.vector.tensor_copy(
                        out=s01T[:], in_=bx[:].rearrange("p a b -> p (a b)"))
                    s01Tf = s01T
                    # a_dst per edge via tiny matmuls
                    adt_ps = psd.tile([P, kg, 4], f32, tag="adt", space="PSUM")
                    o = 0
                    for bk in range(NBANK):
                        for t in tiles:
                            for k in range(int(K[t, bk])):
                                nc.tensor.matmul(
                                    out=adt_ps[:, o, :],
                                    lhsT=s01Tf[:, o * P:(o + 1) * P],
                                    rhs=adst_sb[:, t, :],
                                    start=True, stop=True)
                                o += 1
                    # a_src per edge: reuse msg buffer as temp
                    msg = psm.tile([P, kg, 132], bf16, tag="msg")
                    nc.vector.tensor_tensor(
                        out=msg[:, :, 0:128],
                        in0=gt[:],
                        in1=atts_sb[:, None, :].to_broadcast([P, kg, P]),
                        op=OP.mult)
                    asr = psm.tile([P, kg, 4], f32, tag="asr")
                    nc.vector.tensor_reduce(
                        out=asr[:],
                        in_=msg[:, :, 0:128].rearrange("p k (h c) -> p k h c", h=H),
                        axis=mybir.AxisListType.X, op=OP.add)
                    zt = psm.tile([P, kg, 4], f32, tag="zt")
                    nc.vector.tensor_tensor(out=zt[:], in0=asr[:], in1=adt_ps[:],
                                            op=OP.add)
                    lr = psm.tile([P, kg * 4], f32, tag="lr")
                    nc.vector.scalar_tensor_tensor(
                        out=lr[:], in0=zt[:].rearrange("p k f -> p (k f)"),
                        scalar=NEG, in1=zt[:].rearrange("p k f -> p (k f)"),
                        op0=OP.mult, op1=OP.max)
                    wb = psm.tile([P, kg * 4], bf16, tag="wb")
                    nc.scalar.activation(out=wb[:], in_=lr[:], func=AF.Exp)
                    nc.vector.tensor_tensor(
                        out=msg[:, :, 0:128].rearrange("p k (h c) -> p k h c", h=H),
                        in0=gt[:].rearrange("p k (h c) -> p k h c", h=H),
                        in1=wb[:].rearrange("p (k h) -> p k h", h=H)[:, :, :, None]
                            .to_broadcast([P, kg, H, C]),
                        op=OP.mult)
                    nc.vector.tensor_copy(
                        out=msg[:, :, 128:132],
                        in_=wb[:].rearrange("p (k f) -> p k f", f=4))
                    # scatter into per-tile accumulators
                    accs = {}
                    for ti, k, first, last in pl["scatter"]:
                        if first:
                            acc_t = psb.tile([P, 132], f32, tag="acc",
                                             space="PSUM")
                            accs[ti] = acc_t
                        nc.tensor.matmul(out=accs[ti][:], lhsT=s01g[:, k, :],
                                         rhs=msg[:, k, 0:132],
                                         start=first, stop=last)
                    # epilogue per tile
                    for ti, t in enumerate(tiles):
                        accp = accs[ti]
                        rec = psm.tile([P, 4], f32, tag="rec")
                        nc.vector.reciprocal(out=rec[:], in_=accp[:, 128:132])
                        rec2 = psm.tile([P, 4], f32, tag="rec2")
                        nc.vector.tensor_scalar_mul(out=rec2[:], in0=rec[:],
                                                    scalar1=1.0 / H)
                        tmp = psm.tile([P, P], f32, tag="tmp")
                        nc.vector.tensor_tensor(
                            out=tmp[:].rearrange("p (h c) -> p h c", h=H),
                            in0=accp[:, 0:128].rearrange("p (h c) -> p h c", h=H),
                            in1=rec2[:, :, None].to_broadcast([P, H, C]),
                            op=OP.mult)
                        hsum = psm.tile([P, C], f32, tag="hsum")
                        nc.vector.tensor_reduce(
                            out=hsum[:],
                            in_=tmp[:].rearrange("p (h c) -> p c h", h=H),
                            axis=mybir.AxisListType.X, op=OP.add)
                        badd = psm.tile([P, C], f32, tag="badd")
                        nc.vector.tensor_add(out=badd[:], in0=hsum[:],
                                             in1=bias_sb[:])
                        nc.vector.tensor_scalar_max(out=outall_sb[:, t, :],
                                                    in0=badd[:], scalar1=0.0)
                nc.sync.dma_start(out=out_v[:, :, :], in_=outall_sb[:])
    nc.compile()
    return nc


def prepare(x, edge_index, W, att_src, att_dst, bias):
    x = np.asarray(x, np.float32)
    W = np.asarray(W, np.float32)
    att_src = np.asarray(att_src, np.float32)
    att_dst = np.asarray(att_dst, np.float32)
    bias = np.asarray(bias, np.float32)

    wa = np.zeros((P, 132), np.float32)
    wa[:, :128] = W
    for hh in range(H):
        wa[:, 128 + hh] = W[:, hh * C:(hh + 1) * C] @ att_dst[hh]
    wa_bf = wa.astype(BF16)

    x_pad = np.zeros((NPAD, P), np.float32)
    x_pad[:N] = x
    x_bf = x_pad.astype(BF16)

    K, idx_maps, slot_maps, bm_maps = _prep_edges(np.asarray(edge_index))
    nc = _build_program(K)

    iota_np = np.tile(np.arange(P, dtype=np.float32)[None, :], (P, 1)).astype(BF16)
    # att_src replicated: atts[p, c] = att_src[c // 32, c % 32]
    atts_np = np.tile(att_src.reshape(-1)[None, :], (P, 1)).astype(BF16)
    bias_rep = np.tile(bias[None, :], (P, 1)).astype(np.float32)

    in_maps = []
    for c in range(NCORE):
        xc = np.roll(x_bf, -c * B, axis=0)
        in_maps.append({
            "xbf": np.ascontiguousarray(xc),
            "waug": wa_bf,
            "idx_all": idx_maps[c],
            "slot_all": slot_maps[c],
            "bm_all": bm_maps[c],
            "iota_in": iota_np,
            "atts_in": atts_np,
            "bias_in": bias_rep,
        })
    return nc, in_maps


def kernel(x, edge_index, W, att_src, att_dst, bias):
    nc, in_maps = prepare(x, edge_index, W, att_src, att_dst, bias)
    res = run_bass_kernel_spmd(nc, in_maps, list(range(NCORE)))
    out = np.empty((NPAD, C), np.float32)
    for c in range(NCORE):
        out[c * B:(c + 1) * B] = res.results[c]["out"]
    return out[:N]


# revision 15
# speedup vs baseline: 1.3246x; 1.2574x over previous
"""GATConv forward on 8 Trainium2 NeuronCores (Bass/Tile).

Strategy: destination-node sharding. Host sorts edges by dst, assigns each
core a contiguous dst range (12544 nodes = 98 tiles of 128). Node ids are
cyclically renumbered per core so every core's local nodes are 0..12543 and
the SPMD program is identical across cores; all per-core variation lives in
the input data.

Phase A: T = x @ [W | W@att_src | 0.5*W@att_dst] (bf16, 512B-stride rows,
136 used cols) written block-by-block to HBM; 0.5*a_dst per local node kept
in SBUF (slot-partitioned, fp16).

Phase B: dst tiles are bin-packed into groups of 3. Per group, per src-bank,
dma_gathers of <=1024 rows (SWDGE ring limit) fetch [h | a_src | ...] rows
edge-partitioned. Two host-packed uint16 bitmaps are bit-expanded on DVE
with (x & (1<<b)) << (14-b), whose u16 result 0x4000 bitcast to fp16 is
exactly 2.0 -- giving scaled one-hot matrices with no cast op:
  s01g [edge, slot] (scatter lhsT; the uniform 2x cancels in the softmax
  ratio) and s01T [slot, edge] (a_dst expansion lhsT; 2x cancels against
  the host-halved att_dst).
a_dst per edge comes from tiny PE matmuls s01T^T @ adst; w = exp(lrelu(
a_src+a_dst)); messages [w*h | w] scatter into a per-group [128,3,132] PSUM
accumulator via one-hot matmuls; epilogue computes relu(mean_h num/den +
bias) batched per group.
"""
import sys

sys.path.insert(0, "/opt/trn_rl_repo")
import numpy as np
import ml_dtypes

import concourse.bass as bass
import concourse.mybir as mybir
import concourse.tile as tile
from concourse.bass_utils import run_bass_kernel_spmd
from concourse import bacc

BF16 = ml_dtypes.bfloat16
P = 128
N = 100000
NPAD = 100352          # 784 tiles of 128; 8 cores x 12544
NCORE = 8
B = NPAD // NCORE      # 12544 local nodes per core
TPC = B // P           # 98 tiles per core
NTILE = NPAD // P      # 784 global row tiles
BANK = 32768
NBANK = (NPAD + BANK - 1) // BANK  # 4
NEG = 0.2
H, C = 4, 32
G = 3                  # dst tiles per group (bin-packed)
EL = 256               # T row stride in elements (512B); 136 cols used


def _prep_edges(edge_index):
    src0 = edge_index[0].astype(np.int64)
    dst0 = edge_index[1].astype(np.int64)
    loops = np.arange(NPAD, dtype=np.int64)
    src = np.concatenate([src0, loops])
    dst = np.concatenate([dst0, loops])

    per_core = []
    cnts = np.zeros((NCORE, TPC * NBANK), np.int64)
    for c in range(NCORE):
        lo, hi = c * B, (c + 1) * B
        sel = (dst >= lo) & (dst < hi)
        d = dst[sel] - lo
        s = (src[sel] - lo) % NPAD
        t = d >> 7
        sl = d & 127
        bk = s >> 15
        il = s & (BANK - 1)
        q = t * NBANK + bk
        per_core.append((q, il, sl))
        cnts[c] = np.bincount(q, minlength=TPC * NBANK)

    K = np.ceil(cnts.max(axis=0) / P).astype(np.int64).reshape(TPC, NBANK)

    # bin-pack tiles into groups of G, balancing total chunks per group
    w = K.sum(axis=1)
    order = np.argsort(-w, kind="stable")
    ngrp = (TPC + G - 1) // G
    bins = [[] for _ in range(ngrp)]
    load = np.zeros(ngrp, np.int64)
    for t in order:
        cand = [b for b in range(ngrp) if len(bins[b]) < G]
        b = min(cand, key=lambda i: load[i])
        bins[b].append(int(t))
        load[b] += w[t]
    groups = [sorted(b) for b in bins]

    # global chunk layout: (group, bank, tile, chunk)
    qorder = []
    for tiles in groups:
        for bk in range(NBANK):
            for t in tiles:
                qorder.append(t * NBANK + bk)
    qorder = np.array(qorder, np.int64)
    sz_by_q = (K.reshape(-1) * P)
    sz_in_order = sz_by_q[qorder]
    goff_in_order = np.zeros(len(qorder) + 1, np.int64)
    np.cumsum(sz_in_order, out=goff_in_order[1:])
    tot = int(goff_in_order[-1])
    qoff = np.zeros(TPC * NBANK, np.int64)
    qoff[qorder] = goff_in_order[:-1]

    TOTC = tot // P
    idx_maps, bm_maps = [], []
    for c in range(NCORE):
        q, il, sl = per_core[c]
        cnt = cnts[c]
        start = np.zeros(TPC * NBANK + 1, np.int64)
        np.cumsum(cnt, out=start[1:])
        order_e = np.argsort(q, kind="stable")
        qs = q[order_e]
        rank = np.arange(len(qs)) - start[qs]
        pos = qoff[qs] + rank               # global padded edge position
        idx_pad = np.zeros(tot, np.int16)
        idx_pad[pos] = il[order_e].astype(np.int16)

        # idx table: per chunk [16, 8] wrap -> [16, tot/16], replicated to 128
        idx16 = np.ascontiguousarray(
            idx_pad.reshape(TOTC, 8, 16).transpose(2, 0, 1).reshape(16, TOTC * 8))
        idx128 = np.ascontiguousarray(np.tile(idx16, (8, 1)))

        cc = pos >> 7                       # chunk of each real edge
        lane = pos & 127                    # partition lane within chunk
        slv = sl[order_e]                   # slot (dst & 127) of each edge
        # bmg[e, cc*8 + slot//16] bit slot%16  (edge-partitioned, bits=slot)
        bmg = np.zeros((P, TOTC * 8), np.uint16)
        np.bitwise_or.at(bmg, (lane, cc * 8 + (slv >> 4)),
                         (1 << (slv & 15)).astype(np.uint16))
        # bmt[slot, cc*8 + lane//16] bit lane%16 (slot-partitioned, bits=edge)
        bmt = np.zeros((P, TOTC * 8), np.uint16)
        np.bitwise_or.at(bmt, (slv, cc * 8 + (lane >> 4)),
                         (1 << (lane & 15)).astype(np.uint16))
        idx_maps.append(idx128)
        bm_maps.append((bmg, bmt))
    return K, groups, idx_maps, bm_maps


def _plan(K, groups):
    """Per-group program metadata (shared across cores)."""
    plan = []
    cc = 0
    for tiles in groups:
        gathers = []   # (bank, nch, chunk_off_in_group); nch <= 8
        off = 0
        for bk in range(NBANK):
            nch_bk = int(K[tiles, bk].sum())
            for p0 in range(0, nch_bk, 8):
                gathers.append((bk, min(8, nch_bk - p0), off + p0))
            off += nch_bk
        kg = off
        # chunk -> owning tile (local index), in (bank, tile, chunk) order
        owner = []
        for bk in range(NBANK):
            for ti, t in enumerate(tiles):
                owner += [ti] * int(K[t, bk])
        # scatter order: tile-major so accumulation groups don't interleave
        scatter = []   # (t_local, chunk idx, start, stop)
        for ti, t in enumerate(tiles):
            ks = [k for k in range(kg) if owner[k] == ti]
            for i, k in enumerate(ks):
                scatter.append((ti, k, i == 0, i == len(ks) - 1))
        plan.append(dict(tiles=tiles, gathers=gathers, kg=kg, cc0=cc,
                         owner=owner, scatter=scatter))
        cc += kg
    return plan, cc


def _interleave_bitmaps(plan, TOTC, bm_maps):
    out = []
    for bmg, bmt in bm_maps:
        m = np.zeros((P, TOTC * 16), np.uint16)
        for pl in plan:
            c0, kg = pl["cc0"], pl["kg"]
            m[:, c0 * 16:c0 * 16 + kg * 8] = bmg[:, c0 * 8:(c0 + kg) * 8]
            m[:, c0 * 16 + kg * 8:(c0 + kg) * 16] = bmt[:, c0 * 8:(c0 + kg) * 8]
        out.append(m)
    return out


def _build_program(K, groups):
    plan, TOTC = _plan(K, groups)
    TOT16 = TOTC * 8
    f32, bf16, fp16, i16, u16 = (mybir.dt.float32, mybir.dt.bfloat16,
                                 mybir.dt.float16, mybir.dt.int16,
                                 mybir.dt.uint16)
    AF = mybir.ActivationFunctionType
    OP = mybir.AluOpType

    nc = bacc.Bacc("TRN2", target_bir_lowering=False, debug=False,
                   num_devices=NCORE)
    xbf = nc.dram_tensor("xbf", [NPAD, P], bf16, kind="ExternalInput")
    waug = nc.dram_tensor("waug", [P, 136], bf16, kind="ExternalInput")
    idx_all = nc.dram_tensor("idx_all", [P, TOT16], i16, kind="ExternalInput")
    bm_all = nc.dram_tensor("bm_all", [P, TOTC * 16], u16, kind="ExternalInput")
    bias_in = nc.dram_tensor("bias_in", [P, C], f32, kind="ExternalInput")
    T = nc.dram_tensor("T", [NPAD, EL], bf16)
    out_d = nc.dram_tensor("out", [B, C], f32, kind="ExternalOutput")

    Tv = T[:, :].rearrange("(t p) e -> p t e", p=P)         # [128, 784, 256]
    out_v = out_d[:, :].rearrange("(t p) c -> p t c", p=P)  # [128, 98, 32]

    with tile.TileContext(nc) as tc:
        with tc.tile_pool(name="const", bufs=1) as cp:
            waug_sb = cp.tile([P, 136], bf16)
            nc.sync.dma_start(out=waug_sb[:], in_=waug[:, :])
            bias_sb = cp.tile([P, C], f32)
            nc.sync.dma_start(out=bias_sb[:], in_=bias_in[:, :])
            idx_sb = cp.tile([P, TOT16], i16)
            nc.sync.dma_start(out=idx_sb[:], in_=idx_all[:, :])
            adst_sb = cp.tile([P, TPC, 4], fp16)
            outall_sb = cp.tile([P, TPC, C], f32)

            # ------------- Phase A: T = x @ waug, a_dst table --------------
            NB = NPAD // 1024
            with tc.tile_pool(name="pa", bufs=3) as pa, \
                 tc.tile_pool(name="psa", bufs=2, space="PSUM") as psa:
                for rb in range(NB):
                    xT = pa.tile([P, 1024], bf16, tag="xT")
                    nc.sync.dma_start(out=xT[:],
                                      in_=xbf[rb * 1024:(rb + 1) * 1024, :],
                                      transpose=True)
                    Tb = pa.tile([P, 8, 136], bf16, tag="Tb")
                    for half in range(2):
                        ps = psa.tile([P, 4, 512], f32, tag="psA", space="PSUM")
                        for i in range(4):
                            nc.tensor.matmul(
                                out=ps[:, i, 0:136],
                                lhsT=xT[:, (half * 4 + i) * P:(half * 4 + i + 1) * P],
                                rhs=waug_sb[:], start=True, stop=True)
                        nc.scalar.activation(
                            out=Tb[:, half * 4:(half + 1) * 4, :],
                            in_=ps[:, :, 0:136], func=AF.Copy)
                        t0 = rb * 8 + half * 4
                        if t0 < TPC:
                            nloc = min(4, TPC - t0)
                            nc.scalar.activation(
                                out=adst_sb[:, t0:t0 + nloc, :],
                                in_=ps[:, 0:nloc, 132:136], func=AF.Copy)
                    nc.scalar.dma_start(out=Tv[:, rb * 8:(rb + 1) * 8, 0:136],
                                        in_=Tb[:])

            tc.strict_bb_all_engine_barrier()

            # ------------- Phase B: gather / scatter -----------------------
            with tc.tile_pool(name="pgt", bufs=2) as pgt, \
                 tc.tile_pool(name="pbm", bufs=2) as pbm, \
                 tc.tile_pool(name="ptr", bufs=1) as ptr, \
                 tc.tile_pool(name="psm", bufs=2) as psm, \
                 tc.tile_pool(name="psb", bufs=2, space="PSUM") as psb, \
                 tc.tile_pool(name="psd", bufs=2, space="PSUM") as psd:
                for g, pl in enumerate(plan):
                    kg, cc0, tiles = pl["kg"], pl["cc0"], pl["tiles"]
                    k8 = kg * 8
                    gt = pgt.tile([P, kg, EL], bf16, tag="gath")
                    for bk, nch, off in pl["gathers"]:
                        rows = min(BANK, NPAD - bk * BANK)
                        nc.gpsimd.dma_gather(
                            out_ap=gt[:, off:off + nch, :],
                            in_ap=T[bk * BANK:bk * BANK + rows, :],
                            idxs_ap=idx_sb[:, (cc0 + off) * 8:(cc0 + off + nch) * 8],
                            num_idxs=nch * P, num_idxs_reg=nch * P,
                            elem_size=EL)
                    bm_t = pbm.tile([P, 2 * k8], u16, tag="bm")
                    nc.scalar.dma_start(out=bm_t[:],
                                        in_=bm_all[:, cc0 * 16:cc0 * 16 + 2 * k8])
                    # bit-expand to fp16 2.0-valued one-hots (word-major so
                    # the slot/edge index is contiguous for matmul lhsT):
                    # ((x & (1<<b)) << (14-b)) = 0x4000 = fp16(2.0)
                    bx = ptr.tile([P, 2 * k8, 16], u16, tag="bx")
                    for b in range(16):
                        sh = (OP.logical_shift_left if b <= 14
                              else OP.logical_shift_right)
                        nc.vector.tensor_scalar(
                            out=bx[:, :, b], in0=bm_t[:],
                            scalar1=1 << b, scalar2=abs(14 - b),
                            op0=OP.bitwise_and, op1=sh)
                    bxf = bx[:].bitcast(fp16)
                    s01g = bxf[:, 0:k8, :].rearrange("p (k w) b -> p k (w b)",
                                                     w=8)
                    s01t = bxf[:, k8:2 * k8, :].rearrange(
                        "p (k w) b -> p k (w b)", w=8)
                    # a_dst per edge: adt[e, k, h] = sum_s s01T[s, k, e]*adst[s]
                    adt_ps = psd.tile([P, kg, 4], f32, tag="adt", space="PSUM")
                    for k in range(kg):
                        nc.tensor.matmul(
                            out=adt_ps[:, k, :], lhsT=s01t[:, k, :],
                            rhs=adst_sb[:, tiles[pl["owner"][k]], :],
                            start=True, stop=True)
                    # z = a_src + a_dst; w = exp(leakyrelu(z))
                    zt = psm.tile([P, kg, 4], fp16, tag="zt")
                    nc.vector.tensor_tensor(out=zt[:], in0=gt[:, :, 128:132],
                                            in1=adt_ps[:], op=OP.add)
                    lr = psm.tile([P, kg * 4], fp16, tag="lr")
                    nc.vector.scalar_tensor_tensor(
                        out=lr[:], in0=zt[:].rearrange("p k f -> p (k f)"),
                        scalar=NEG, in1=zt[:].rearrange("p k f -> p (k f)"),
                        op0=OP.mult, op1=OP.max)
                    wb = psm.tile([P, kg, 4], fp16, tag="wb")
                    nc.scalar.activation(
                        out=wb[:].rearrange("p k f -> p (k f)"), in_=lr[:],
                        func=AF.Exp)
                    # msg = [w*h | w]
                    msg = psm.tile([P, kg, 132], fp16, tag="msg")
                    nc.vector.scalar_tensor_tensor(
                        out=msg[:, :, 0:128].rearrange("p k (h c) -> p k h c", h=H),
                        in0=gt[:, :, 0:128].rearrange("p k (h c) -> p k h c", h=H),
                        scalar=1.0,
                        in1=wb[:, :, :, None].to_broadcast([P, kg, H, C]),
                        op0=OP.mult, op1=OP.mult)
                    nc.vector.tensor_copy(out=msg[:, :, 128:132], in_=wb[:])
                    # scatter into per-group accumulator [128, G, 132]
                    acc = psb.tile([P, G, 132], f32, tag="acc", space="PSUM")
                    for ti, k, first, last in pl["scatter"]:
                        nc.tensor.matmul(out=acc[:, ti, :], lhsT=s01g[:, k, :],
                                         rhs=msg[:, k, 0:132],
                                         start=first, stop=last)
                    # epilogue, batched over the group's tiles
                    ng = len(tiles)
                    den = psm.tile([P, G, 4], f32, tag="den")
                    nc.vector.reciprocal(out=den[:, 0:ng, :],
                                         in_=acc[:, 0:ng, 128:132])
                    tmp = psm.tile([P, G, P], f32, tag="tmp")
                    nc.vector.tensor_tensor(
                        out=tmp[:, 0:ng, :].rearrange("p g (h c) -> p g h c", h=H),
                        in0=acc[:, 0:ng, 0:128].rearrange("p g (h c) -> p g h c", h=H),
                        in1=den[:, 0:ng, :, None].to_broadcast([P, ng, H, C]),
                        op=OP.mult)
                    hsum = psm.tile([P, G, C], f32, tag="hsum")
                    nc.vector.tensor_reduce(
                        out=hsum[:, 0:ng, :],
                        in_=tmp[:, 0:ng, :].rearrange("p g (h c) -> p g c h", h=H),
                        axis=mybir.AxisListType.X, op=OP.add)
                    badd = psm.tile([P, G, C], f32, tag="badd")
                    nc.vector.scalar_tensor_tensor(
                        out=badd[:, 0:ng, :], in0=hsum[:, 0:ng, :],
                        scalar=1.0 / H,
                        in1=bias_sb[:, None, :].to_broadcast([P, ng, C]),
                        op0=OP.mult, op1=OP.add)
                    for ti, t in enumerate(tiles):
                        nc.vector.tensor_scalar_max(
                            out=outall_sb[:, t, :], in0=badd[:, ti, :],
                            scalar1=0.0)
                nc.sync.dma_start(out=out_v[:, :, :], in_=outall_sb[:])
    nc.compile()
    return nc


def prepare(x, edge_index, W, att_src, att_dst, bias):
    x = np.asarray(x, np.float32)
    W = np.asarray(W, np.float32)
    att_src = np.asarray(att_src, np.float32)
    att_dst = np.asarray(att_dst, np.float32)
    bias = np.asarray(bias, np.float32)

    wa = np.zeros((P, 136), np.float32)
    wa[:, :128] = W
    for hh in range(H):
        wa[:, 128 + hh] = W[:, hh * C:(hh + 1) * C] @ att_src[hh]
        # one-hot entries are 2.0 (bit shifted to fp16 exponent); halve a_dst
        wa[:, 132 + hh] = 0.5 * (W[:, hh * C:(hh + 1) * C] @ att_dst[hh])
    wa_bf = wa.astype(BF16)

    x_pad = np.zeros((NPAD, P), np.float32)
    x_pad[:N] = x
    x_bf = x_pad.astype(BF16)

    K, groups, idx_maps, bm_maps = _prep_edges(np.asarray(edge_index))
    plan, TOTC = _plan(K, groups)
    bmi_maps = _interleave_bitmaps(plan, TOTC, bm_maps)
    nc = _build_program(K, groups)

    bias_rep = np.tile(bias[None, :], (P, 1)).astype(np.float32)

    in_maps = []
    for c in range(NCORE):
        xc = np.roll(x_bf, -c * B, axis=0)
        in_maps.append({
            "xbf": np.ascontiguousarray(xc),
            "waug": wa_bf,
            "idx_all": idx_maps[c],
            "bm_all": bmi_maps[c],
            "bias_in": bias_rep,
        })
    return nc, in_maps


def kernel(x, edge_index, W, att_src, att_dst, bias):
    nc, in_maps = prepare(x, edge_index, W, att_src, att_dst, bias)
    res = run_bass_kernel_spmd(nc, in_maps, list(range(NCORE)))
    out = np.empty((NPAD, C), np.float32)
    for c in range(NCORE):
        out[c * B:(c + 1) * B] = res.results[c]["out"]
    return out[:N]


# revision 19
# speedup vs baseline: 1.4546x; 1.0981x over previous
"""GATConv forward on 8 Trainium2 NeuronCores (Bass/Tile).

Strategy: destination-node sharding. Host sorts edges by dst, assigns each
core a contiguous dst range (12544 nodes = 98 tiles of 128). Node ids are
cyclically renumbered per core so every core's local nodes are 0..12543 and
the SPMD program is identical across cores; all per-core variation lives in
the input data.

Phase A: per 128-node tile, [h | a_src | 0.5*a_dst] = x @ waug on the PE;
rows packed into a 512B-row HBM table T as [h bf16 (256B) | a_src fp16
(8B) | pad]; 0.5*a_dst for local tiles kept in SBUF (slot-partitioned,
fp16).

Phase B: dst tiles are bin-packed into groups of 3 and processed in a
software pipeline so each engine's in-order queue never head-of-line
blocks on another engine:
  iteration i issues  gather(i)+streams(i+1) | bitexpand(i) | a_dst(i)
  [PE] | z/w/msg(i-1) [DVE+Act] | scatter(i-1) [PE] | epilogue(i-2).
dma_gathers are capped at 8 chunks (1024-descriptor SWDGE ring limit).
One-hot matrices come from host-packed uint16 bitmaps bit-expanded with
(x & (1<<b)) << (14-b), whose u16 result 0x4000 bitcast to fp16 is exactly
2.0 (scale cancels in the softmax ratio; att_dst is pre-halved). a_dst per
edge = tiny PE matmuls s01T^T @ adst; w = exp(lrelu(a_src + a_dst));
messages [w*h | w] scatter into a per-group [128,3,132] PSUM accumulator;
epilogue relu(mean_h num/den + bias).
"""
import sys

sys.path.insert(0, "/opt/trn_rl_repo")
import numpy as np
import ml_dtypes

import concourse.bass as bass
import concourse.mybir as mybir
import concourse.tile as tile
from concourse.bass_utils import run_bass_kernel_spmd
from concourse import bacc

BF16 = ml_dtypes.bfloat16
P = 128
N = 100000
NPAD = 100352          # 784 tiles of 128; 8 cores x 12544
NCORE = 8
B = NPAD // NCORE      # 12544 local nodes per core
TPC = B // P           # 98 tiles per core
NTILE = NPAD // P      # 784 global row tiles
BANK = 32768
NBANK = (NPAD + BANK - 1) // BANK  # 4
NEG = 0.2
H, C = 4, 32
G = 3                  # dst tiles per group (bin-packed)
EW = 256               # T row width in u16 elements (512B)


def _prep_edges(edge_index):
    src0 = edge_index[0].astype(np.int64)
    dst0 = edge_index[1].astype(np.int64)
    loops = np.arange(NPAD, dtype=np.int64)
    src = np.concatenate([src0, loops])
    dst = np.concatenate([dst0, loops])

    per_core = []
    cnts = np.zeros((NCORE, TPC * NBANK), np.int64)
    for c in range(NCORE):
        lo, hi = c * B, (c + 1) * B
        sel = (dst >= lo) & (dst < hi)
        d = dst[sel] - lo
        s = (src[sel] - lo) % NPAD
        t = d >> 7
        sl = d & 127
        bk = s >> 15
        il = s & (BANK - 1)
        q = t * NBANK + bk
        per_core.append((q, il, sl))
        cnts[c] = np.bincount(q, minlength=TPC * NBANK)

    K = np.ceil(cnts.max(axis=0) / P).astype(np.int64).reshape(TPC, NBANK)

    # bin-pack tiles into groups of G, balancing total chunks per group
    w = K.sum(axis=1)
    order = np.argsort(-w, kind="stable")
    ngrp = (TPC + G - 1) // G
    bins = [[] for _ in range(ngrp)]
    load = np.zeros(ngrp, np.int64)
    for t in order:
        cand = [b for b in range(ngrp) if len(bins[b]) < G]
        b = min(cand, key=lambda i: load[i])
        bins[b].append(int(t))
        load[b] += w[t]
    groups = [sorted(b) for b in bins]

    # global chunk layout: (group, bank, tile, chunk)
    qorder = []
    for tiles in groups:
        for bk in range(NBANK):
            for t in tiles:
                qorder.append(t * NBANK + bk)
    qorder = np.array(qorder, np.int64)
    sz_by_q = (K.reshape(-1) * P)
    sz_in_order = sz_by_q[qorder]
    goff_in_order = np.zeros(len(qorder) + 1, np.int64)
    np.cumsum(sz_in_order, out=goff_in_order[1:])
    tot = int(goff_in_order[-1])
    qoff = np.zeros(TPC * NBANK, np.int64)
    qoff[qorder] = goff_in_order[:-1]

    TOTC = tot // P
    idx_maps, bm_maps = [], []
    for c in range(NCORE):
        q, il, sl = per_core[c]
        cnt = cnts[c]
        start = np.zeros(TPC * NBANK + 1, np.int64)
        np.cumsum(cnt, out=start[1:])
        order_e = np.argsort(q, kind="stable")
        qs = q[order_e]
        rank = np.arange(len(qs)) - start[qs]
        pos = qoff[qs] + rank               # global padded edge position
        idx_pad = np.zeros(tot, np.int16)
        idx_pad[pos] = il[order_e].astype(np.int16)

        # idx table: per chunk [16, 8] wrap -> [16, tot/16], replicated to 128
        idx16 = np.ascontiguousarray(
            idx_pad.reshape(TOTC, 8, 16).transpose(2, 0, 1).reshape(16, TOTC * 8))
        idx128 = np.ascontiguousarray(np.tile(idx16, (8, 1)))

        cc = pos >> 7                       # chunk of each real edge
        lane = pos & 127                    # partition lane within chunk
        slv = sl[order_e]                   # slot (dst & 127) of each edge
        # bmg[e, cc*8 + slot//16] bit slot%16  (edge-partitioned, bits=slot)
        bmg = np.zeros((P, TOTC * 8), np.uint16)
        np.bitwise_or.at(bmg, (lane, cc * 8 + (slv >> 4)),
                         (1 << (slv & 15)).astype(np.uint16))
        # bmt[slot, cc*8 + lane//16] bit lane%16 (slot-partitioned, bits=edge)
        bmt = np.zeros((P, TOTC * 8), np.uint16)
        np.bitwise_or.at(bmt, (slv, cc * 8 + (lane >> 4)),
                         (1 << (lane & 15)).astype(np.uint16))
        idx_maps.append(idx128)
        bm_maps.append((bmg, bmt))
    return K, groups, idx_maps, bm_maps


def _plan(K, groups):
    """Per-group program metadata (shared across cores)."""
    plan = []
    cc = 0
    for tiles in groups:
        gathers = []   # (bank, nch, chunk_off_in_group); nch <= 8
        off = 0
        for bk in range(NBANK):
            nch_bk = int(K[tiles, bk].sum())
            for p0 in range(0, nch_bk, 8):
                gathers.append((bk, min(8, nch_bk - p0), off + p0))
            off += nch_bk
        kg = off
        # chunk -> owning tile (local index), in (bank, tile, chunk) order
        owner = []
        for bk in range(NBANK):
            for ti, t in enumerate(tiles):
                owner += [ti] * int(K[t, bk])
        # scatter order: tile-major so accumulation groups don't interleave
        scatter = []   # (t_local, chunk idx, start, stop)
        for ti, t in enumerate(tiles):
            ks = [k for k in range(kg) if owner[k] == ti]
            for i, k in enumerate(ks):
                scatter.append((ti, k, i == 0, i == len(ks) - 1))
        plan.append(dict(tiles=tiles, gathers=gathers, kg=kg, cc0=cc,
                         owner=owner, scatter=scatter))
        cc += kg
    return plan, cc


def _interleave_bitmaps(plan, TOTC, bm_maps):
    out = []
    for bmg, bmt in bm_maps:
        m = np.zeros((P, TOTC * 16), np.uint16)
        for pl in plan:
            c0, kg = pl["cc0"], pl["kg"]
            m[:, c0 * 16:c0 * 16 + kg * 8] = bmg[:, c0 * 8:(c0 + kg) * 8]
            m[:, c0 * 16 + kg * 8:(c0 + kg) * 16] = bmt[:, c0 * 8:(c0 + kg) * 8]
        out.append(m)
    return out


def _build_program(K, groups):
    plan, TOTC = _plan(K, groups)
    NG = len(plan)
    TOT16 = TOTC * 8
    f32, bf16, fp16, fp8, i16, u16 = (
        mybir.dt.float32, mybir.dt.bfloat16, mybir.dt.float16,
        mybir.dt.float8e4, mybir.dt.int16, mybir.dt.uint16)
    AF = mybir.ActivationFunctionType
    OP = mybir.AluOpType

    nc = bacc.Bacc("TRN2", target_bir_lowering=False, debug=False,
                   num_devices=NCORE)
    xbf = nc.dram_tensor("xbf", [NPAD, P], bf16, kind="ExternalInput")
    waug = nc.dram_tensor("waug", [P, 136], bf16, kind="ExternalInput")
    idx_all = nc.dram_tensor("idx_all", [P, TOT16], i16, kind="ExternalInput")
    bm_all = nc.dram_tensor("bm_all", [P, TOTC * 16], u16, kind="ExternalInput")
    bias_in = nc.dram_tensor("bias_in", [P, C], f32, kind="ExternalInput")
    T = nc.dram_tensor("T", [NPAD, EW], u16)
    out_d = nc.dram_tensor("out", [B, C], f32, kind="ExternalOutput")

    Tv = T[:, :].rearrange("(t p) e -> p t e", p=P)         # [128, 784, 128]
    out_v = out_d[:, :].rearrange("(t p) c -> p t c", p=P)  # [128, 98, 32]

    with tile.TileContext(nc) as tc:
        with tc.tile_pool(name="const", bufs=1) as cp:
            waug_sb = cp.tile([P, 136], bf16)
            nc.sync.dma_start(out=waug_sb[:], in_=waug[:, :])
            bias_sb = cp.tile([P, C], f32)
            nc.sync.dma_start(out=bias_sb[:], in_=bias_in[:, :])
            adst_sb = cp.tile([P, TPC, 4], fp16)
            outall_sb = cp.tile([P, TPC, C], f32)

            # ------------- Phase A: T = x @ waug, a_dst table --------------
            NB = NPAD // 1024
            with tc.tile_pool(name="pa", bufs=4) as pa, \
                 tc.tile_pool(name="psa", bufs=2, space="PSUM") as psa:
                for rb in range(NB):
                    xT = pa.tile([P, 1024], bf16, tag="xT")
                    nc.sync.dma_start(out=xT[:],
                                      in_=xbf[rb * 1024:(rb + 1) * 1024, :],
                                      transpose=True)
                    Tb = pa.tile([P, 8, EW], u16, tag="Tb")
                    for half in range(2):
                        ps = psa.tile([P, 4, 512], f32, tag="psA", space="PSUM")
                        for i in range(4):
                            nc.tensor.matmul(
                                out=ps[:, i, 0:136],
                                lhsT=xT[:, (half * 4 + i) * P:(half * 4 + i + 1) * P],
                                rhs=waug_sb[:], start=True, stop=True)
                        h4 = slice(half * 4, (half + 1) * 4)
                        nc.scalar.activation(
                            out=Tb[:, h4, 0:128].bitcast(bf16),
                            in_=ps[:, :, 0:128], func=AF.Copy)
                        nc.scalar.activation(
                            out=Tb[:, h4, 128:132].bitcast(fp16),
                            in_=ps[:, :, 128:132], func=AF.Copy)
                        t0 = rb * 8 + half * 4
                        if t0 < TPC:
                            nloc = min(4, TPC - t0)
                            nc.scalar.activation(
                                out=adst_sb[:, t0:t0 + nloc, :],
                                in_=ps[:, 0:nloc, 132:136], func=AF.Copy)
                    nc.gpsimd.dma_start(out=Tv[:, rb * 8:(rb + 1) * 8, 0:132],
                                        in_=Tb[:, :, 0:132])

            tc.strict_bb_all_engine_barrier()

            # ------------- Phase B: software-pipelined groups --------------
            with tc.tile_pool(name="pgt", bufs=2) as pgt, \
                 tc.tile_pool(name="pix", bufs=2) as pix, \
                 tc.tile_pool(name="pbm", bufs=2) as pbm, \
                 tc.tile_pool(name="ptr", bufs=2) as ptr, \
                 tc.tile_pool(name="pt1", bufs=1) as pt1, \
                 tc.tile_pool(name="psm", bufs=2) as psm, \
                 tc.tile_pool(name="pms", bufs=1) as pms, \
                 tc.tile_pool(name="psb", bufs=3, space="PSUM") as psb, \
                 tc.tile_pool(name="psd", bufs=2, space="PSUM") as psd:
                st = {}  # live per-group tiles

                def issue_streams(g):
                    pl = plan[g]
                    kg, cc0 = pl["kg"], pl["cc0"]
                    idx_t = pix.tile([P, kg * 8], i16, tag="idx")
                    nc.scalar.dma_start(
                        out=idx_t[:], in_=idx_all[:, cc0 * 8:(cc0 + kg) * 8])
                    bm_t = pbm.tile([P, 2, kg * 8], u16, tag="bm")
                    nc.scalar.dma_start(
                        out=bm_t[:].rearrange("p a b -> p (a b)"),
                        in_=bm_all[:, cc0 * 16:(cc0 + kg) * 16])
                    st[g] = {"idx": idx_t, "bm": bm_t}

                def issue_gather(g):
                    pl = plan[g]
                    kg = pl["kg"]
                    gt = pgt.tile([P, kg, EW], u16, tag="gath")
                    idx_t = st[g]["idx"]
                    for bk, nch, off in pl["gathers"]:
                        rows = min(BANK, NPAD - bk * BANK)
                        nc.gpsimd.dma_gather(
                            out_ap=gt[:, off:off + nch, :],
                            in_ap=T[bk * BANK:bk * BANK + rows, :],
                            idxs_ap=idx_t[:, off * 8:(off + nch) * 8],
                            num_idxs=nch * P, num_idxs_reg=nch * P,
                            elem_size=EW)
                    st[g]["gt"] = gt

                def issue_bitexp(g):
                    pl = plan[g]
                    kg = pl["kg"]
                    k8 = kg * 8
                    bm_t = st[g]["bm"]
                    bxg = ptr.tile([P, k8, 16], u16, tag="bxg")
                    bxt = pt1.tile([P, k8, 16], u16, tag="bxt")
                    for b in range(16):
                        sh = (OP.logical_shift_left if b <= 14
                              else OP.logical_shift_right)
                        nc.vector.tensor_scalar(
                            out=bxt[:, :, b],
                            in0=bm_t[:, 1], scalar1=1 << b, scalar2=abs(14 - b),
                            op0=OP.bitwise_and, op1=sh)
                        nc.vector.tensor_scalar(
                            out=bxg[:, :, b],
                            in0=bm_t[:, 0], scalar1=1 << b, scalar2=abs(14 - b),
                            op0=OP.bitwise_and, op1=sh)
                    st[g]["s01g"] = bxg[:].bitcast(fp16).rearrange(
                        "p (k w) b -> p k (w b)", w=8)
                    st[g]["s01t"] = bxt[:].bitcast(fp16).rearrange(
                        "p (k w) b -> p k (w b)", w=8)

                def issue_adt(g):
                    pl = plan[g]
                    kg, tiles = pl["kg"], pl["tiles"]
                    s01t = st[g]["s01t"]
                    adt_ps = psd.tile([P, kg, 4], f32, tag="adt", space="PSUM")
                    for k in range(kg):
                        nc.tensor.matmul(
                            out=adt_ps[:, k, :], lhsT=s01t[:, k, :],
                            rhs=adst_sb[:, tiles[pl["owner"][k]], :],
                            start=True, stop=True)
                    st[g]["adt"] = adt_ps

                def issue_msg(g):
                    pl = plan[g]
                    kg = pl["kg"]
                    gt = st[g]["gt"]
                    hview = gt[:, :, 0:128].bitcast(bf16)    # [P, kg, 128]
                    aview = gt[:, :, 128:132].bitcast(fp16)  # [P, kg, 4]
                    zt = psm.tile([P, kg, 4], fp16, tag="zt")
                    nc.vector.tensor_tensor(out=zt[:], in0=aview,
                                            in1=st[g]["adt"][:], op=OP.add)
                    lr = psm.tile([P, kg * 4], fp16, tag="lr")
                    nc.vector.scalar_tensor_tensor(
                        out=lr[:], in0=zt[:].rearrange("p k f -> p (k f)"),
                        scalar=NEG, in1=zt[:].rearrange("p k f -> p (k f)"),
                        op0=OP.mult, op1=OP.max)
                    wb = psm.tile([P, kg, 4], fp16, tag="wb")
                    nc.scalar.activation(
                        out=wb[:].rearrange("p k f -> p (k f)"), in_=lr[:],
                        func=AF.Exp)
                    msg = pms.tile([P, kg, 132], fp16, tag="msg")
                    nc.vector.scalar_tensor_tensor(
                        out=msg[:, :, 0:128].rearrange("p k (h c) -> p k h c", h=H),
                        in0=hview.rearrange("p k (h c) -> p k h c", h=H),
                        scalar=1.0,
                        in1=wb[:, :, :, None].to_broadcast([P, kg, H, C]),
                        op0=OP.mult, op1=OP.mult)
                    nc.vector.tensor_copy(out=msg[:, :, 128:132], in_=wb[:])
                    st[g]["msg"] = msg

                def issue_scatter(g):
                    pl = plan[g]
                    msg, s01g = st[g]["msg"], st[g]["s01g"]
                    acc = psb.tile([P, G, 132], f32, tag="acc", space="PSUM")
                    for ti, k, first, last in pl["scatter"]:
                        nc.tensor.matmul(out=acc[:, ti, :], lhsT=s01g[:, k, :],
                                         rhs=msg[:, k, 0:132],
                                         start=first, stop=last)
                    st[g]["acc"] = acc

                def issue_epilogue(g):
                    pl = plan[g]
                    tiles = pl["tiles"]
                    ng = len(tiles)
                    acc = st[g]["acc"]
                    den = psm.tile([P, G, 4], f32, tag="den")
                    nc.vector.reciprocal(out=den[:, 0:ng, :],
                                         in_=acc[:, 0:ng, 128:132])
                    tmp = psm.tile([P, G, P], f32, tag="tmp")
                    nc.vector.tensor_tensor(
                        out=tmp[:, 0:ng, :].rearrange("p g (h c) -> p g h c", h=H),
                        in0=acc[:, 0:ng, 0:128].rearrange("p g (h c) -> p g h c", h=H),
                        in1=den[:, 0:ng, :, None].to_broadcast([P, ng, H, C]),
                        op=OP.mult)
                    hsum = psm.tile([P, G, C], f32, tag="hsum")
                    nc.vector.tensor_reduce(
                        out=hsum[:, 0:ng, :],
                        in_=tmp[:, 0:ng, :].rearrange("p g (h c) -> p g c h", h=H),
                        axis=mybir.AxisListType.X, op=OP.add)
                    badd = psm.tile([P, G, C], f32, tag="badd")
                    nc.vector.scalar_tensor_tensor(
                        out=badd[:, 0:ng, :], in0=hsum[:, 0:ng, :],
                        scalar=1.0 / H,
                        in1=bias_sb[:, None, :].to_broadcast([P, ng, C]),
                        op0=OP.mult, op1=OP.add)
                    for ti, t in enumerate(tiles):
                        nc.vector.tensor_scalar_max(
                            out=outall_sb[:, t, :], in0=badd[:, ti, :],
                            scalar1=0.0)
                    del st[g]

                issue_streams(0)
                for i in range(NG + 2):
                    if i + 1 < NG:
                        issue_streams(i + 1)
                    if i < NG:
                        issue_gather(i)
                        issue_bitexp(i)
                        issue_adt(i)
                    if 1 <= i <= NG:
                        issue_msg(i - 1)
                        issue_scatter(i - 1)
                    if i >= 2:
                        issue_epilogue(i - 2)
                nc.sync.dma_start(out=out_v[:, :, :], in_=outall_sb[:])
    nc.compile()
    return nc


def prepare(x, edge_index, W, att_src, att_dst, bias):
    x = np.asarray(x, np.float32)
    W = np.asarray(W, np.float32)
    att_src = np.asarray(att_src, np.float32)
    att_dst = np.asarray(att_dst, np.float32)
    bias = np.asarray(bias, np.float32)

    wa = np.zeros((P, 136), np.float32)
    wa[:, :128] = W
    for hh in range(H):
        wa[:, 128 + hh] = W[:, hh * C:(hh + 1) * C] @ att_src[hh]
        # one-hot entries are 2.0 (bit shifted to fp16 exponent); halve a_dst
        wa[:, 132 + hh] = 0.5 * (W[:, hh * C:(hh + 1) * C] @ att_dst[hh])
    wa_bf = wa.astype(BF16)

    x_pad = np.zeros((NPAD, P), np.float32)
    x_pad[:N] = x
    x_bf = x_pad.astype(BF16)

    K, groups, idx_maps, bm_maps = _prep_edges(np.asarray(edge_index))
    plan, TOTC = _plan(K, groups)
    bmi_maps = _interleave_bitmaps(plan, TOTC, bm_maps)
    nc = _build_program(K, groups)

    bias_rep = np.tile(bias[None, :], (P, 1)).astype(np.float32)

    in_maps = []
    for c in range(NCORE):
        xc = np.roll(x_bf, -c * B, axis=0)
        in_maps.append({
            "xbf": np.ascontiguousarray(xc),
            "waug": wa_bf,
            "idx_all": idx_maps[c],
            "bm_all": bmi_maps[c],
            "bias_in": bias_rep,
        })
    return nc, in_maps


def kernel(x, edge_index, W, att_src, att_dst, bias):
    nc, in_maps = prepare(x, edge_index, W, att_src, att_dst, bias)
    res = run_bass_kernel_spmd(nc, in_maps, list(range(NCORE)))
    out = np.empty((NPAD, C), np.float32)
    for c in range(NCORE):
        out[c * B:(c + 1) * B] = res.results[c]["out"]
    return out[:N]


# revision 33
# speedup vs baseline: 2.3255x; 1.5987x over previous
"""GATConv forward on 8 Trainium2 NeuronCores (Bass/Tile).

Strategy: destination-node sharding. Host sorts edges by dst, assigns each
core a contiguous dst range (12544 nodes = 98 tiles of 128). Node ids are
cyclically renumbered per core so every core's local nodes are 0..12543 and
the SPMD program is identical across cores; all per-core variation lives in
the input data.

Phase A: per 128-node tile, [h | a_src | 0.5*a_dst] = x @ waug on the PE;
rows packed into a 512B-row HBM table T as [h bf16 (256B) | a_src fp16
(8B) | pad]; 0.5*a_dst for local tiles kept in SBUF (slot-partitioned,
fp16).

Phase B: dst tiles are bin-packed into groups of 3 and processed in a
software pipeline so each engine's in-order queue never head-of-line
blocks on another engine:
  iteration i issues  gather(i)+streams(i+1) | bitexpand(i) | a_dst(i)
  [PE] | z/w/msg(i-1) [DVE+Act] | scatter(i-1) [PE] | epilogue(i-2).
dma_gathers are capped at 8 chunks (1024-descriptor SWDGE ring limit).
One-hot matrices come from host-packed uint16 bitmaps bit-expanded with
(x & (1<<b)) << (14-b), whose u16 result 0x4000 bitcast to fp16 is exactly
2.0 (scale cancels in the softmax ratio; att_dst is pre-halved). a_dst per
edge = tiny PE matmuls s01T^T @ adst; w = exp(lrelu(a_src + a_dst));
messages [w*h | w] scatter into a per-group [128,3,132] PSUM accumulator;
epilogue relu(mean_h num/den + bias).
"""
import sys

sys.path.insert(0, "/opt/trn_rl_repo")
import numpy as np
import ml_dtypes

import concourse.bass as bass
import concourse.mybir as mybir
import concourse.tile as tile
from concourse.bass_utils import run_bass_kernel_spmd
from concourse import bacc

BF16 = ml_dtypes.bfloat16
P = 128
N = 100000
NPAD = 100352          # 784 tiles of 128; 8 cores x 12544
NCORE = 8
B = NPAD // NCORE      # 12544 local nodes per core
TPC = B // P           # 98 tiles per core
NTILE = NPAD // P      # 784 global row tiles
BANK = 32768
NBANK = (NPAD + BANK - 1) // BANK  # 4
NEG = 0.2
H, C = 4, 32
G = 3                  # dst tiles per group (bin-packed)
EW = 256               # T row width in u16 elements (512B)


def _prep_edges(edge_index):
    src0 = edge_index[0].astype(np.int64)
    dst0 = edge_index[1].astype(np.int64)
    loops = np.arange(NPAD, dtype=np.int64)
    src = np.concatenate([src0, loops])
    dst = np.concatenate([dst0, loops])

    per_core = []
    cnts = np.zeros((NCORE, TPC * NBANK), np.int64)
    for c in range(NCORE):
        lo, hi = c * B, (c + 1) * B
        sel = (dst >= lo) & (dst < hi)
        d = dst[sel] - lo
        s = (src[sel] - lo) % NPAD
        t = d >> 7
        sl = d & 127
        bk = s >> 15
        il = s & (BANK - 1)
        q = t * NBANK + bk
        per_core.append((q, il, sl))
        cnts[c] = np.bincount(q, minlength=TPC * NBANK)

    K = np.ceil(cnts.max(axis=0) / P).astype(np.int64).reshape(TPC, NBANK)

    # bin-pack tiles into groups of G, balancing total chunks per group
    w = K.sum(axis=1)
    order = np.argsort(-w, kind="stable")
    ngrp = (TPC + G - 1) // G
    bins = [[] for _ in range(ngrp)]
    load = np.zeros(ngrp, np.int64)
    for t in order:
        cand = [b for b in range(ngrp) if len(bins[b]) < G]
        b = min(cand, key=lambda i: load[i])
        bins[b].append(int(t))
        load[b] += w[t]
    groups = [sorted(b) for b in bins]

    # global chunk layout: (group, bank, tile, chunk)
    qorder = []
    for tiles in groups:
        for bk in range(NBANK):
            for t in tiles:
                qorder.append(t * NBANK + bk)
    qorder = np.array(qorder, np.int64)
    sz_by_q = (K.reshape(-1) * P)
    sz_in_order = sz_by_q[qorder]
    goff_in_order = np.zeros(len(qorder) + 1, np.int64)
    np.cumsum(sz_in_order, out=goff_in_order[1:])
    tot = int(goff_in_order[-1])
    qoff = np.zeros(TPC * NBANK, np.int64)
    qoff[qorder] = goff_in_order[:-1]

    TOTC = tot // P
    idx_maps, bm_maps = [], []
    for c in range(NCORE):
        q, il, sl = per_core[c]
        cnt = cnts[c]
        start = np.zeros(TPC * NBANK + 1, np.int64)
        np.cumsum(cnt, out=start[1:])
        order_e = np.argsort(q, kind="stable")
        qs = q[order_e]
        rank = np.arange(len(qs)) - start[qs]
        pos = qoff[qs] + rank               # global padded edge position
        idx_pad = np.zeros(tot, np.int16)
        idx_pad[pos] = il[order_e].astype(np.int16)

        # idx table: per chunk [16, 8] wrap -> [16, tot/16], replicated to 128
        idx16 = np.ascontiguousarray(
            idx_pad.reshape(TOTC, 8, 16).transpose(2, 0, 1).reshape(16, TOTC * 8))
        idx128 = np.ascontiguousarray(np.tile(idx16, (8, 1)))

        cc = pos >> 7                       # chunk of each real edge
        lane = pos & 127                    # partition lane within chunk
        slv = sl[order_e]                   # slot (dst & 127) of each edge
        # bmg[e, cc*8 + slot//16] bit slot%16  (edge-partitioned, bits=slot)
        bmg = np.zeros((P, TOTC * 8), np.uint16)
        np.bitwise_or.at(bmg, (lane, cc * 8 + (slv >> 4)),
                         (1 << (slv & 15)).astype(np.uint16))
        # bmt[slot, cc*8 + lane//16] bit lane%16 (slot-partitioned, bits=edge)
        bmt = np.zeros((P, TOTC * 8), np.uint16)
        np.bitwise_or.at(bmt, (slv, cc * 8 + (lane >> 4)),
                         (1 << (lane & 15)).astype(np.uint16))
        idx_maps.append(idx128)
        bm_maps.append((bmg, bmt))
    return K, groups, idx_maps, bm_maps


def _plan(K, groups):
    """Per-group program metadata (shared across cores)."""
    plan = []
    cc = 0
    for tiles in groups:
        gathers = []   # (bank, nch, chunk_off_in_group); nch <= 8
        off = 0
        for bk in range(NBANK):
            nch_bk = int(K[tiles, bk].sum())
            for p0 in range(0, nch_bk, 8):
                gathers.append((bk, min(8, nch_bk - p0), off + p0))
            off += nch_bk
        kg = off
        # chunk -> owning tile (local index), in (bank, tile, chunk) order
        owner = []
        for bk in range(NBANK):
            for ti, t in enumerate(tiles):
                owner += [ti] * int(K[t, bk])
        # scatter order: tile-major so accumulation groups don't interleave
        scatter = []   # (t_local, chunk idx, start, stop)
        for ti, t in enumerate(tiles):
            ks = [k for k in range(kg) if owner[k] == ti]
            for i, k in enumerate(ks):
                scatter.append((ti, k, i == 0, i == len(ks) - 1))
        plan.append(dict(tiles=tiles, gathers=gathers, kg=kg, cc0=cc,
                         owner=owner, scatter=scatter))
        cc += kg
    return plan, cc


def _interleave_bitmaps(plan, TOTC, bm_maps):
    out = []
    for bmg, bmt in bm_maps:
        m = np.zeros((P, TOTC * 16), np.uint16)
        for pl in plan:
            c0, kg = pl["cc0"], pl["kg"]
            m[:, c0 * 16:c0 * 16 + kg * 8] = bmg[:, c0 * 8:(c0 + kg) * 8]
            m[:, c0 * 16 + kg * 8:(c0 + kg) * 16] = bmt[:, c0 * 8:(c0 + kg) * 8]
        out.append(m)
    return out


def _build_program(K, groups):
    plan, TOTC = _plan(K, groups)
    NG = len(plan)
    TOT16 = TOTC * 8
    f32, bf16, fp16, fp8, i16, u16 = (
        mybir.dt.float32, mybir.dt.bfloat16, mybir.dt.float16,
        mybir.dt.float8e4, mybir.dt.int16, mybir.dt.uint16)
    AF = mybir.ActivationFunctionType
    OP = mybir.AluOpType

    nc = bacc.Bacc("TRN2", target_bir_lowering=False, debug=False,
                   num_devices=NCORE)
    xbf = nc.dram_tensor("xbf", [NPAD, P], bf16, kind="ExternalInput")
    waug = nc.dram_tensor("waug", [P, 136], bf16, kind="ExternalInput")
    idx_all = nc.dram_tensor("idx_all", [P, TOT16], i16, kind="ExternalInput")
    bm_all = nc.dram_tensor("bm_all", [P, TOTC * 16], u16, kind="ExternalInput")
    bias_in = nc.dram_tensor("bias_in", [P, C], f32, kind="ExternalInput")
    T = nc.dram_tensor("T", [NPAD, EW], u16)
    out_d = nc.dram_tensor("out", [B, C], f32, kind="ExternalOutput")

    Tv = T[:, :].rearrange("(t p) e -> p t e", p=P)         # [128, 784, 128]
    out_v = out_d[:, :].rearrange("(t p) c -> p t c", p=P)  # [128, 98, 32]

    with tile.TileContext(nc) as tc:
        with tc.tile_pool(name="const", bufs=1) as cp:
            waug_sb = cp.tile([P, 136], bf16)
            nc.sync.dma_start(out=waug_sb[:], in_=waug[:, :])
            bias_sb = cp.tile([P, C], f32)
            nc.sync.dma_start(out=bias_sb[:], in_=bias_in[:, :])
            adst_sb = cp.tile([P, TPC, 4], fp16)
            outall_sb = cp.tile([P, TPC, C], f32)

            # ------------- Phase A: T = x @ waug, a_dst table --------------
            # Chunked into few, large DMAs: the tile scheduler serializes
            # any DMA against the next one with a ~5.5us bubble, so one
            # 8192-row transpose-in and one 64-tile T-write-out per chunk
            # bounds that cost to ~13 bubbles total. T writes go through
            # HWDGE (Act queue) -- SWDGE's 1024-descriptor ring cannot take
            # an 8192-descriptor DMA.
            CH = 16384
            NCHUNK = (NPAD + CH - 1) // CH  # 7 (last chunk 2048 rows)
            with tc.tile_pool(name="pax", bufs=2) as pax, \
                 tc.tile_pool(name="pat", bufs=2) as pat, \
                 tc.tile_pool(name="psa", bufs=2, space="PSUM") as psa:
                for ck in range(NCHUNK):
                    r0 = ck * CH
                    nrows = min(CH, NPAD - r0)
                    nt = nrows // P                  # tiles in chunk (<=64)
                    xT = pax.tile([P, CH], bf16, tag="xT")
                    nc.sync.dma_start(out=xT[:, 0:nrows],
                                      in_=xbf[r0:r0 + nrows, :],
                                      transpose=True)
                    Tb = pat.tile([P, CH // P, 132], u16, tag="Tb")
                    for q in range(nt // 4):
                        ps = psa.tile([P, 4, 512], f32, tag="psA",
                                      space="PSUM")
                        for i in range(4):
                            nc.tensor.matmul(
                                out=ps[:, i, 0:136],
                                lhsT=xT[:, (q * 4 + i) * P:(q * 4 + i + 1) * P],
                                rhs=waug_sb[:], start=True, stop=True)
                        q4 = slice(q * 4, (q + 1) * 4)
                        if q % 2 == 0:
                            nc.scalar.activation(
                                out=Tb[:, q4, 0:128].bitcast(bf16),
                                in_=ps[:, :, 0:128], func=AF.Copy)
                        else:
                            nc.vector.tensor_copy(
                                out=Tb[:, q4, 0:128].bitcast(bf16),
                                in_=ps[:, :, 0:128])
                        nc.scalar.activation(
                            out=Tb[:, q4, 128:132].bitcast(fp16),
                            in_=ps[:, :, 128:132], func=AF.Copy)
                        t0 = r0 // P + q * 4
                        if t0 < TPC:
                            nloc = min(4, TPC - t0)
                            nc.scalar.activation(
                                out=adst_sb[:, t0:t0 + nloc, :],
                                in_=ps[:, 0:nloc, 132:136], func=AF.Copy)
                    nc.scalar.dma_start(
                        out=Tv[:, r0 // P:r0 // P + nt, 0:132],
                        in_=Tb[:, 0:nt, :])

            tc.strict_bb_all_engine_barrier()

            # ------------- Phase B: software-pipelined groups --------------
            with tc.tile_pool(name="pgt", bufs=2) as pgt, \
                 tc.tile_pool(name="pix", bufs=2) as pix, \
                 tc.tile_pool(name="pbm", bufs=2) as pbm, \
                 tc.tile_pool(name="ptr", bufs=2) as ptr, \
                 tc.tile_pool(name="pt1", bufs=1) as pt1, \
                 tc.tile_pool(name="psm", bufs=2) as psm, \
                 tc.tile_pool(name="pms", bufs=1) as pms, \
                 tc.tile_pool(name="psb", bufs=3, space="PSUM") as psb, \
                 tc.tile_pool(name="psd", bufs=2, space="PSUM") as psd:
                st = {}  # live per-group tiles
                nregs = {}
                for pl_ in plan:
                    for _, nch_, _ in pl_["gathers"]:
                        nregs.setdefault(nch_ * P, None)
                for v in sorted(nregs):
                    nregs[v] = nc.gpsimd.to_reg(v)

                def issue_streams(g):
                    pl = plan[g]
                    kg, cc0 = pl["kg"], pl["cc0"]
                    idx_t = pix.tile([P, kg * 8], i16, tag="idx")
                    nc.scalar.dma_start(
                        out=idx_t[:], in_=idx_all[:, cc0 * 8:(cc0 + kg) * 8])
                    bm_t = pbm.tile([P, 2, kg * 8], u16, tag="bm")
                    nc.scalar.dma_start(
                        out=bm_t[:].rearrange("p a b -> p (a b)"),
                        in_=bm_all[:, cc0 * 16:(cc0 + kg) * 16])
                    st[g] = {"idx": idx_t, "bm": bm_t}

                def issue_gather(g):
                    pl = plan[g]
                    kg = pl["kg"]
                    gt = pgt.tile([P, kg, EW], u16, tag="gath")
                    idx_t = st[g]["idx"]
                    for bk, nch, off in pl["gathers"]:
                        rows = min(BANK, NPAD - bk * BANK)
                        nc.gpsimd.dma_gather(
                            out_ap=gt[:, off:off + nch, :],
                            in_ap=T[bk * BANK:bk * BANK + rows, :],
                            idxs_ap=idx_t[:, off * 8:(off + nch) * 8],
                            num_idxs=nch * P, num_idxs_reg=nregs[nch * P],
                            elem_size=EW)
                    st[g]["gt"] = gt

                def issue_bitexp(g):
                    pl = plan[g]
                    kg = pl["kg"]
                    k8 = kg * 8
                    bm_t = st[g]["bm"]
                    bxg = ptr.tile([P, k8, 16], u16, tag="bxg")
                    bxt = pt1.tile([P, k8, 16], u16, tag="bxt")
                    for b in range(16):
                        sh = (OP.logical_shift_left if b <= 14
                              else OP.logical_shift_right)
                        nc.vector.tensor_scalar(
                            out=bxt[:, :, b],
                            in0=bm_t[:, 1], scalar1=1 << b, scalar2=abs(14 - b),
                            op0=OP.bitwise_and, op1=sh)
                        nc.vector.tensor_scalar(
                            out=bxg[:, :, b],
                            in0=bm_t[:, 0], scalar1=1 << b, scalar2=abs(14 - b),
                            op0=OP.bitwise_and, op1=sh)
                    st[g]["s01g"] = bxg[:].bitcast(fp16).rearrange(
                        "p (k w) b -> p k (w b)", w=8)
                    st[g]["s01t"] = bxt[:].bitcast(fp16).rearrange(
                        "p (k w) b -> p k (w b)", w=8)

                def issue_adt(g):
                    pl = plan[g]
                    kg, tiles = pl["kg"], pl["tiles"]
                    s01t = st[g]["s01t"]
                    adt_ps = psd.tile([P, kg, 4], f32, tag="adt", space="PSUM")
                    for k in range(kg):
                        nc.tensor.matmul(
                            out=adt_ps[:, k, :], lhsT=s01t[:, k, :],
                            rhs=adst_sb[:, tiles[pl["owner"][k]], :],
                            start=True, stop=True)
                    st[g]["adt"] = adt_ps

                def issue_msg(g):
                    pl = plan[g]
                    kg = pl["kg"]
                    gt = st[g]["gt"]
                    hview = gt[:, :, 0:128].bitcast(bf16)    # [P, kg, 128]
                    aview = gt[:, :, 128:132].bitcast(fp16)  # [P, kg, 4]
                    zt = psm.tile([P, kg, 4], fp16, tag="zt")
                    nc.vector.tensor_tensor(out=zt[:], in0=aview,
                                            in1=st[g]["adt"][:], op=OP.add)
                    lr = psm.tile([P, kg * 4], fp16, tag="lr")
                    nc.vector.scalar_tensor_tensor(
                        out=lr[:], in0=zt[:].rearrange("p k f -> p (k f)"),
                        scalar=NEG, in1=zt[:].rearrange("p k f -> p (k f)"),
                        op0=OP.mult, op1=OP.max)
                    wb = psm.tile([P, kg, 4], fp16, tag="wb")
                    nc.scalar.activation(
                        out=wb[:].rearrange("p k f -> p (k f)"), in_=lr[:],
                        func=AF.Exp)
                    wbx = pms.tile([P, kg, H, C], fp16, tag="wbx")
                    nc.scalar.activation(
                        out=wbx[:],
                        in_=wb[:, :, :, None].to_broadcast([P, kg, H, C]),
                        func=AF.Copy)
                    msg = pms.tile([P, kg, 132], fp16, tag="msg")
                    nc.vector.tensor_tensor(
                        out=msg[:, :, 0:128],
                        in0=hview,
                        in1=wbx[:].rearrange("p k h c -> p (k h c)")
                            .rearrange("p (k f) -> p k f", k=kg),
                        op=OP.mult)
                    nc.scalar.activation(out=msg[:, :, 128:132], in_=wb[:],
                                         func=AF.Copy)
                    st[g]["msg"] = msg

                def issue_scatter(g):
                    pl = plan[g]
                    msg, s01g = st[g]["msg"], st[g]["s01g"]
                    acc = psb.tile([P, G, 132], f32, tag="acc", space="PSUM")
                    for ti, k, first, last in pl["scatter"]:
                        nc.tensor.matmul(out=acc[:, ti, :], lhsT=s01g[:, k, :],
                                         rhs=msg[:, k, 0:132],
                                         start=first, stop=last)
                    st[g]["acc"] = acc

                def issue_epilogue(g):
                    pl = plan[g]
                    tiles = pl["tiles"]
                    ng = len(tiles)
                    acc = st[g]["acc"]
                    den = psm.tile([P, G, 4], f32, tag="den")
                    nc.vector.reciprocal(out=den[:, 0:ng, :],
                                         in_=acc[:, 0:ng, 128:132])
                    tmp = psm.tile([P, G, P], f32, tag="tmp")
                    nc.vector.tensor_tensor(
                        out=tmp[:, 0:ng, :].rearrange("p g (h c) -> p g h c", h=H),
                        in0=acc[:, 0:ng, 0:128].rearrange("p g (h c) -> p g h c", h=H),
                        in1=den[:, 0:ng, :, None].to_broadcast([P, ng, H, C]),
                        op=OP.mult)
                    hsum = psm.tile([P, G, C], f32, tag="hsum")
                    nc.vector.tensor_reduce(
                        out=hsum[:, 0:ng, :],
                        in_=tmp[:, 0:ng, :].rearrange("p g (h c) -> p g c h", h=H),
                        axis=mybir.AxisListType.X, op=OP.add)
                    badd = psm.tile([P, G, C], f32, tag="badd")
                    nc.vector.scalar_tensor_tensor(
                        out=badd[:, 0:ng, :], in0=hsum[:, 0:ng, :],
                        scalar=1.0 / H,
                        in1=bias_sb[:, None, :].to_broadcast([P, ng, C]),
                        op0=OP.mult, op1=OP.add)
                    for ti, t in enumerate(tiles):
                        nc.vector.tensor_scalar_max(
                            out=outall_sb[:, t, :], in0=badd[:, ti, :],
                            scalar1=0.0)
                    del st[g]

                issue_streams(0)
                for i in range(NG + 3):
                    if i + 1 < NG:
                        issue_streams(i + 1)
                    if i > NG:
                        pass
                    if i < NG:
                        issue_gather(i)
                        issue_bitexp(i)
                        issue_adt(i)
                    if 1 <= i <= NG:
                        issue_msg(i - 1)
                        issue_scatter(i - 1)
                    if i >= 3:
                        issue_epilogue(i - 3)
                nc.sync.dma_start(out=out_v[:, :, :], in_=outall_sb[:])
    nc.compile()
    return nc


def prepare(x, edge_index, W, att_src, att_dst, bias):
    x = np.asarray(x, np.float32)
    W = np.asarray(W, np.float32)
    att_src = np.asarray(att_src, np.float32)
    att_dst = np.asarray(att_dst, np.float32)
    bias = np.asarray(bias, np.float32)

    wa = np.zeros((P, 136), np.float32)
    wa[:, :128] = W
    for hh in range(H):
        wa[:, 128 + hh] = W[:, hh * C:(hh + 1) * C] @ att_src[hh]
        # one-hot entries are 2.0 (bit shifted to fp16 exponent); halve a_dst
        wa[:, 132 + hh] = 0.5 * (W[:, hh * C:(hh + 1) * C] @ att_dst[hh])
    wa_bf = wa.astype(BF16)

    x_pad = np.zeros((NPAD, P), np.float32)
    x_pad[:N] = x
    x_bf = x_pad.astype(BF16)

    K, groups, idx_maps, bm_maps = _prep_edges(np.asarray(edge_index))
    plan, TOTC = _plan(K, groups)
    bmi_maps = _interleave_bitmaps(plan, TOTC, bm_maps)
    nc = _build_program(K, groups)

    bias_rep = np.tile(bias[None, :], (P, 1)).astype(np.float32)

    in_maps = []
    for c in range(NCORE):
        xc = np.roll(x_bf, -c * B, axis=0)
        in_maps.append({
            "xbf": np.ascontiguousarray(xc),
            "waug": wa_bf,
            "idx_all": idx_maps[c],
            "bm_all": bmi_maps[c],
            "bias_in": bias_rep,
        })
    return nc, in_maps


def kernel(x, edge_index, W, att_src, att_dst, bias):
    nc, in_maps = prepare(x, edge_index, W, att_src, att_dst, bias)
    res = run_bass_kernel_spmd(nc, in_maps, list(range(NCORE)))
    out = np.empty((NPAD, C), np.float32)
    for c in range(NCORE):
        out[c * B:(c + 1) * B] = res.results[c]["out"]
    return out[:N]


# revision 34
# speedup vs baseline: 2.3360x; 1.0045x over previous
"""GATConv forward on 8 Trainium2 NeuronCores (Bass/Tile).

Strategy: destination-node sharding. Host sorts edges by dst, assigns each
core a contiguous dst range (12544 nodes = 98 tiles of 128). Node ids are
cyclically renumbered per core so every core's local nodes are 0..12543 and
the SPMD program is identical across cores; all per-core variation lives in
the input data.

Phase A: per 128-node tile, [h | a_src | 0.5*a_dst] = x @ waug on the PE;
rows packed into a 512B-row HBM table T as [h bf16 (256B) | a_src fp16
(8B) | pad]; 0.5*a_dst for local tiles kept in SBUF (slot-partitioned,
fp16).

Phase B: dst tiles are bin-packed into groups of 3 and processed in a
software pipeline so each engine's in-order queue never head-of-line
blocks on another engine:
  iteration i issues  gather(i)+streams(i+1) | bitexpand(i) | a_dst(i)
  [PE] | z/w/msg(i-1) [DVE+Act] | scatter(i-1) [PE] | epilogue(i-2).
dma_gathers are capped at 8 chunks (1024-descriptor SWDGE ring limit).
One-hot matrices come from host-packed uint16 bitmaps bit-expanded with
(x & (1<<b)) << (14-b), whose u16 result 0x4000 bitcast to fp16 is exactly
2.0 (scale cancels in the softmax ratio; att_dst is pre-halved). a_dst per
edge = tiny PE matmuls s01T^T @ adst; w = exp(lrelu(a_src + a_dst));
messages [w*h | w] scatter into a per-group [128,3,132] PSUM accumulator;
epilogue relu(mean_h num/den + bias).
"""
import sys

sys.path.insert(0, "/opt/trn_rl_repo")
import numpy as np
import ml_dtypes

import concourse.bass as bass
import concourse.mybir as mybir
import concourse.tile as tile
from concourse.bass_utils import run_bass_kernel_spmd
from concourse import bacc

BF16 = ml_dtypes.bfloat16
P = 128
N = 100000
NPAD = 100352          # 784 tiles of 128; 8 cores x 12544
NCORE = 8
B = NPAD // NCORE      # 12544 local nodes per core
TPC = B // P           # 98 tiles per core
NTILE = NPAD // P      # 784 global row tiles
BANK = 32768
NBANK = (NPAD + BANK - 1) // BANK  # 4
NEG = 0.2
H, C = 4, 32
G = 3                  # dst tiles per group (bin-packed)
EW = 256               # T row width in u16 elements (512B)


def _prep_edges(edge_index):
    src0 = edge_index[0].astype(np.int64)
    dst0 = edge_index[1].astype(np.int64)
    loops = np.arange(NPAD, dtype=np.int64)
    src = np.concatenate([src0, loops])
    dst = np.concatenate([dst0, loops])

    per_core = []
    cnts = np.zeros((NCORE, TPC * NBANK), np.int64)
    for c in range(NCORE):
        lo, hi = c * B, (c + 1) * B
        sel = (dst >= lo) & (dst < hi)
        d = dst[sel] - lo
        s = (src[sel] - lo) % NPAD
        t = d >> 7
        sl = d & 127
        bk = s >> 15
        il = s & (BANK - 1)
        q = t * NBANK + bk
        per_core.append((q, il, sl))
        cnts[c] = np.bincount(q, minlength=TPC * NBANK)

    K = np.ceil(cnts.max(axis=0) / P).astype(np.int64).reshape(TPC, NBANK)

    # bin-pack tiles into groups of G, balancing total chunks per group
    w = K.sum(axis=1)
    order = np.argsort(-w, kind="stable")
    ngrp = (TPC + G - 1) // G
    bins = [[] for _ in range(ngrp)]
    load = np.zeros(ngrp, np.int64)
    for t in order:
        cand = [b for b in range(ngrp) if len(bins[b]) < G]
        b = min(cand, key=lambda i: load[i])
        bins[b].append(int(t))
        load[b] += w[t]
    groups = [sorted(b) for b in bins]

    # global chunk layout: (group, bank, tile, chunk)
    qorder = []
    for tiles in groups:
        for bk in range(NBANK):
            for t in tiles:
                qorder.append(t * NBANK + bk)
    qorder = np.array(qorder, np.int64)
    sz_by_q = (K.reshape(-1) * P)
    sz_in_order = sz_by_q[qorder]
    goff_in_order = np.zeros(len(qorder) + 1, np.int64)
    np.cumsum(sz_in_order, out=goff_in_order[1:])
    tot = int(goff_in_order[-1])
    qoff = np.zeros(TPC * NBANK, np.int64)
    qoff[qorder] = goff_in_order[:-1]

    TOTC = tot // P
    idx_maps, bm_maps = [], []
    for c in range(NCORE):
        q, il, sl = per_core[c]
        cnt = cnts[c]
        start = np.zeros(TPC * NBANK + 1, np.int64)
        np.cumsum(cnt, out=start[1:])
        order_e = np.argsort(q, kind="stable")
        qs = q[order_e]
        rank = np.arange(len(qs)) - start[qs]
        pos = qoff[qs] + rank               # global padded edge position
        idx_pad = np.zeros(tot, np.int16)
        idx_pad[pos] = il[order_e].astype(np.int16)

        # idx table: per chunk [16, 8] wrap -> [16, tot/16], replicated to 128
        idx16 = np.ascontiguousarray(
            idx_pad.reshape(TOTC, 8, 16).transpose(2, 0, 1).reshape(16, TOTC * 8))
        idx128 = np.ascontiguousarray(np.tile(idx16, (8, 1)))

        cc = pos >> 7                       # chunk of each real edge
        lane = pos & 127                    # partition lane within chunk
        slv = sl[order_e]                   # slot (dst & 127) of each edge
        # bmg[e, cc*8 + slot//16] bit slot%16  (edge-partitioned, bits=slot)
        bmg = np.zeros((P, TOTC * 8), np.uint16)
        np.bitwise_or.at(bmg, (lane, cc * 8 + (slv >> 4)),
                         (1 << (slv & 15)).astype(np.uint16))
        # bmt[slot, cc*8 + lane//16] bit lane%16 (slot-partitioned, bits=edge)
        bmt = np.zeros((P, TOTC * 8), np.uint16)
        np.bitwise_or.at(bmt, (slv, cc * 8 + (lane >> 4)),
                         (1 << (lane & 15)).astype(np.uint16))
        idx_maps.append(idx128)
        bm_maps.append((bmg, bmt))
    return K, groups, idx_maps, bm_maps


def _plan(K, groups):
    """Per-group program metadata (shared across cores)."""
    plan = []
    cc = 0
    for tiles in groups:
        gathers = []   # (bank, nch, chunk_off_in_group); nch <= 8
        off = 0
        for bk in range(NBANK):
            nch_bk = int(K[tiles, bk].sum())
            for p0 in range(0, nch_bk, 8):
                gathers.append((bk, min(8, nch_bk - p0), off + p0))
            off += nch_bk
        kg = off
        # chunk -> owning tile (local index), in (bank, tile, chunk) order
        owner = []
        for bk in range(NBANK):
            for ti, t in enumerate(tiles):
                owner += [ti] * int(K[t, bk])
        # scatter order: tile-major so accumulation groups don't interleave
        scatter = []   # (t_local, chunk idx, start, stop)
        for ti, t in enumerate(tiles):
            ks = [k for k in range(kg) if owner[k] == ti]
            for i, k in enumerate(ks):
                scatter.append((ti, k, i == 0, i == len(ks) - 1))
        plan.append(dict(tiles=tiles, gathers=gathers, kg=kg, cc0=cc,
                         owner=owner, scatter=scatter))
        cc += kg
    return plan, cc


def _interleave_bitmaps(plan, TOTC, bm_maps):
    out = []
    for bmg, bmt in bm_maps:
        m = np.zeros((P, TOTC * 16), np.uint16)
        for pl in plan:
            c0, kg = pl["cc0"], pl["kg"]
            m[:, c0 * 16:c0 * 16 + kg * 8] = bmg[:, c0 * 8:(c0 + kg) * 8]
            m[:, c0 * 16 + kg * 8:(c0 + kg) * 16] = bmt[:, c0 * 8:(c0 + kg) * 8]
        out.append(m)
    return out


def _build_program(K, groups):
    plan, TOTC = _plan(K, groups)
    NG = len(plan)
    TOT16 = TOTC * 8
    f32, bf16, fp16, fp8, i16, u16 = (
        mybir.dt.float32, mybir.dt.bfloat16, mybir.dt.float16,
        mybir.dt.float8e4, mybir.dt.int16, mybir.dt.uint16)
    AF = mybir.ActivationFunctionType
    OP = mybir.AluOpType

    nc = bacc.Bacc("TRN2", target_bir_lowering=False, debug=False,
                   num_devices=NCORE)
    xbf = nc.dram_tensor("xbf", [NPAD, P], bf16, kind="ExternalInput")
    waug = nc.dram_tensor("waug", [P, 136], bf16, kind="ExternalInput")
    idx_all = nc.dram_tensor("idx_all", [P, TOT16], i16, kind="ExternalInput")
    bm_all = nc.dram_tensor("bm_all", [P, TOTC * 16], u16, kind="ExternalInput")
    bias_in = nc.dram_tensor("bias_in", [P, C], f32, kind="ExternalInput")
    T = nc.dram_tensor("T", [NPAD, EW], u16)
    out_d = nc.dram_tensor("out", [B, C], f32, kind="ExternalOutput")

    Tv = T[:, :].rearrange("(t p) e -> p t e", p=P)         # [128, 784, 128]
    out_v = out_d[:, :].rearrange("(t p) c -> p t c", p=P)  # [128, 98, 32]

    with tile.TileContext(nc) as tc:
        with tc.tile_pool(name="const", bufs=1) as cp:
            waug_sb = cp.tile([P, 136], bf16)
            nc.sync.dma_start(out=waug_sb[:], in_=waug[:, :])
            bias_sb = cp.tile([P, C], f32)
            nc.sync.dma_start(out=bias_sb[:], in_=bias_in[:, :])
            adst_sb = cp.tile([P, TPC, 4], fp16)
            outall_sb = cp.tile([P, TPC, C], f32)

            # ------------- Phase A: T = x @ waug, a_dst table --------------
            # Chunked into few, large DMAs: the tile scheduler serializes
            # any DMA against the next one with a ~5.5us bubble, so one
            # 8192-row transpose-in and one 64-tile T-write-out per chunk
            # bounds that cost to ~13 bubbles total. T writes go through
            # HWDGE (Act queue) -- SWDGE's 1024-descriptor ring cannot take
            # an 8192-descriptor DMA.
            CH = 16384
            NCHUNK = (NPAD + CH - 1) // CH  # 7 (last chunk 2048 rows)
            with tc.tile_pool(name="pax", bufs=2) as pax, \
                 tc.tile_pool(name="pat", bufs=2) as pat, \
                 tc.tile_pool(name="psa", bufs=2, space="PSUM") as psa:
                for ck in range(NCHUNK):
                    r0 = ck * CH
                    nrows = min(CH, NPAD - r0)
                    nt = nrows // P                  # tiles in chunk (<=64)
                    xT = pax.tile([P, CH], bf16, tag="xT")
                    nc.sync.dma_start(out=xT[:, 0:nrows],
                                      in_=xbf[r0:r0 + nrows, :],
                                      transpose=True)
                    Tb = pat.tile([P, CH // P, 132], u16, tag="Tb")
                    for q in range(nt // 4):
                        ps = psa.tile([P, 4, 512], f32, tag="psA",
                                      space="PSUM")
                        for i in range(4):
                            nc.tensor.matmul(
                                out=ps[:, i, 0:136],
                                lhsT=xT[:, (q * 4 + i) * P:(q * 4 + i + 1) * P],
                                rhs=waug_sb[:], start=True, stop=True)
                        q4 = slice(q * 4, (q + 1) * 4)
                        if q % 2 == 0:
                            nc.scalar.activation(
                                out=Tb[:, q4, 0:128].bitcast(bf16),
                                in_=ps[:, :, 0:128], func=AF.Copy)
                        else:
                            nc.vector.tensor_copy(
                                out=Tb[:, q4, 0:128].bitcast(bf16),
                                in_=ps[:, :, 0:128])
                        nc.scalar.activation(
                            out=Tb[:, q4, 128:132].bitcast(fp16),
                            in_=ps[:, :, 128:132], func=AF.Copy)
                        t0 = r0 // P + q * 4
                        if t0 < TPC:
                            nloc = min(4, TPC - t0)
                            nc.scalar.activation(
                                out=adst_sb[:, t0:t0 + nloc, :],
                                in_=ps[:, 0:nloc, 132:136], func=AF.Copy)
                    nc.scalar.dma_start(
                        out=Tv[:, r0 // P:r0 // P + nt, 0:132],
                        in_=Tb[:, 0:nt, :])

            tc.strict_bb_all_engine_barrier()

            # ------------- Phase B: software-pipelined groups --------------
            with tc.tile_pool(name="pgt", bufs=2) as pgt, \
                 tc.tile_pool(name="pix", bufs=2) as pix, \
                 tc.tile_pool(name="pbm", bufs=2) as pbm, \
                 tc.tile_pool(name="ptr", bufs=2) as ptr, \
                 tc.tile_pool(name="pt1", bufs=1) as pt1, \
                 tc.tile_pool(name="psm", bufs=2) as psm, \
                 tc.tile_pool(name="pms", bufs=1) as pms, \
                 tc.tile_pool(name="psb", bufs=3, space="PSUM") as psb, \
                 tc.tile_pool(name="psd", bufs=2, space="PSUM") as psd:
                st = {}  # live per-group tiles
                nregs = {}
                for pl_ in plan:
                    for _, nch_, _ in pl_["gathers"]:
                        nregs.setdefault(nch_ * P, None)
                for v in sorted(nregs):
                    nregs[v] = nc.gpsimd.to_reg(v)

                def issue_streams(g):
                    pl = plan[g]
                    kg, cc0 = pl["kg"], pl["cc0"]
                    idx_t = pix.tile([P, kg * 8], i16, tag="idx")
                    nc.scalar.dma_start(
                        out=idx_t[:], in_=idx_all[:, cc0 * 8:(cc0 + kg) * 8])
                    bm_t = pbm.tile([P, 2, kg * 8], u16, tag="bm")
                    nc.scalar.dma_start(
                        out=bm_t[:].rearrange("p a b -> p (a b)"),
                        in_=bm_all[:, cc0 * 16:(cc0 + kg) * 16])
                    st[g] = {"idx": idx_t, "bm": bm_t}

                def issue_gather(g):
                    pl = plan[g]
                    kg = pl["kg"]
                    gt = pgt.tile([P, kg, EW], u16, tag="gath")
                    idx_t = st[g]["idx"]
                    for bk, nch, off in pl["gathers"]:
                        rows = min(BANK, NPAD - bk * BANK)
                        nc.gpsimd.dma_gather(
                            out_ap=gt[:, off:off + nch, :],
                            in_ap=T[bk * BANK:bk * BANK + rows, :],
                            idxs_ap=idx_t[:, off * 8:(off + nch) * 8],
                            num_idxs=nch * P, num_idxs_reg=nregs[nch * P],
                            elem_size=EW)
                    st[g]["gt"] = gt

                def issue_bitexp(g):
                    pl = plan[g]
                    kg = pl["kg"]
                    k8 = kg * 8
                    bm_t = st[g]["bm"]
                    bxg = ptr.tile([P, k8, 16], u16, tag="bxg")
                    bxt = pt1.tile([P, k8, 16], u16, tag="bxt")
                    for b in range(16):
                        sh = (OP.logical_shift_left if b <= 14
                              else OP.logical_shift_right)
                        nc.vector.tensor_scalar(
                            out=bxt[:, :, b],
                            in0=bm_t[:, 1], scalar1=1 << b, scalar2=abs(14 - b),
                            op0=OP.bitwise_and, op1=sh)
                        nc.vector.tensor_scalar(
                            out=bxg[:, :, b],
                            in0=bm_t[:, 0], scalar1=1 << b, scalar2=abs(14 - b),
                            op0=OP.bitwise_and, op1=sh)
                    st[g]["s01g"] = bxg[:].bitcast(fp16).rearrange(
                        "p (k w) b -> p k (w b)", w=8)
                    st[g]["s01t"] = bxt[:].bitcast(fp16).rearrange(
                        "p (k w) b -> p k (w b)", w=8)

                def issue_adt(g):
                    pl = plan[g]
                    kg, tiles = pl["kg"], pl["tiles"]
                    s01t = st[g]["s01t"]
                    adt_ps = psd.tile([P, kg, 4], f32, tag="adt", space="PSUM")
                    for k in range(kg):
                        nc.tensor.matmul(
                            out=adt_ps[:, k, :], lhsT=s01t[:, k, :],
                            rhs=adst_sb[:, tiles[pl["owner"][k]], :],
                            start=True, stop=True)
                    st[g]["adt"] = adt_ps

                def issue_msg(g):
                    pl = plan[g]
                    kg = pl["kg"]
                    gt = st[g]["gt"]
                    hview = gt[:, :, 0:128].bitcast(bf16)    # [P, kg, 128]
                    aview = gt[:, :, 128:132].bitcast(fp16)  # [P, kg, 4]
                    zt = psm.tile([P, kg, 4], fp16, tag="zt")
                    nc.vector.tensor_tensor(out=zt[:], in0=aview,
                                            in1=st[g]["adt"][:], op=OP.add)
                    lr = psm.tile([P, kg * 4], fp16, tag="lr")
                    nc.vector.scalar_tensor_tensor(
                        out=lr[:], in0=zt[:].rearrange("p k f -> p (k f)"),
                        scalar=NEG, in1=zt[:].rearrange("p k f -> p (k f)"),
                        op0=OP.mult, op1=OP.max)
                    wb = psm.tile([P, kg, 4], fp16, tag="wb")
                    nc.scalar.activation(
                        out=wb[:].rearrange("p k f -> p (k f)"), in_=lr[:],
                        func=AF.Exp)
                    wbx = pms.tile([P, kg, H, C], fp16, tag="wbx")
                    nc.scalar.activation(
                        out=wbx[:],
                        in_=wb[:, :, :, None].to_broadcast([P, kg, H, C]),
                        func=AF.Copy)
                    msg = pms.tile([P, kg, 132], fp16, tag="msg")
                    nc.vector.tensor_tensor(
                        out=msg[:, :, 0:128],
                        in0=hview,
                        in1=wbx[:].rearrange("p k h c -> p (k h c)")
                            .rearrange("p (k f) -> p k f", k=kg),
                        op=OP.mult)
                    nc.scalar.activation(out=msg[:, :, 128:132], in_=wb[:],
                                         func=AF.Copy)
                    st[g]["msg"] = msg

                def issue_scatter(g):
                    pl = plan[g]
                    msg, s01g = st[g]["msg"], st[g]["s01g"]
                    acc = psb.tile([P, G, 132], f32, tag="acc", space="PSUM")
                    for ti, k, first, last in pl["scatter"]:
                        nc.tensor.matmul(out=acc[:, ti, :], lhsT=s01g[:, k, :],
                                         rhs=msg[:, k, 0:132],
                                         start=first, stop=last)
                    st[g]["acc"] = acc

                def issue_epilogue(g):
                    pl = plan[g]
                    tiles = pl["tiles"]
                    ng = len(tiles)
                    acc = st[g]["acc"]
                    den = psm.tile([P, G, 4], f32, tag="den")
                    nc.vector.reciprocal(out=den[:, 0:ng, :],
                                         in_=acc[:, 0:ng, 128:132])
                    tmp = psm.tile([P, G, P], f32, tag="tmp")
                    nc.vector.tensor_tensor(
                        out=tmp[:, 0:ng, :].rearrange("p g (h c) -> p g h c", h=H),
                        in0=acc[:, 0:ng, 0:128].rearrange("p g (h c) -> p g h c", h=H),
                        in1=den[:, 0:ng, :, None].to_broadcast([P, ng, H, C]),
                        op=OP.mult)
                    hsum = psm.tile([P, G, C], f32, tag="hsum")
                    nc.vector.tensor_reduce(
                        out=hsum[:, 0:ng, :],
                        in_=tmp[:, 0:ng, :].rearrange("p g (h c) -> p g c h", h=H),
                        axis=mybir.AxisListType.X, op=OP.add)
                    badd = psm.tile([P, G, C], f32, tag="badd")
                    nc.vector.scalar_tensor_tensor(
                        out=badd[:, 0:ng, :], in0=hsum[:, 0:ng, :],
                        scalar=1.0 / H,
                        in1=bias_sb[:, None, :].to_broadcast([P, ng, C]),
                        op0=OP.mult, op1=OP.add)
                    for ti, t in enumerate(tiles):
                        nc.vector.tensor_scalar_max(
                            out=outall_sb[:, t, :], in0=badd[:, ti, :],
                            scalar1=0.0)
                    del st[g]

                issue_streams(0)
                for i in range(NG + 2):
                    if i + 1 < NG:
                        issue_streams(i + 1)
                    if i < NG:
                        issue_gather(i)
                        issue_bitexp(i)
                        issue_adt(i)
                    if 1 <= i <= NG:
                        issue_msg(i - 1)
                        issue_scatter(i - 1)
                    if i >= 2:
                        issue_epilogue(i - 2)
                nc.sync.dma_start(out=out_v[:, :, :], in_=outall_sb[:])
    nc.compile()
    return nc


def prepare(x, edge_index, W, att_src, att_dst, bias):
    x = np.asarray(x, np.float32)
    W = np.asarray(W, np.float32)
    att_src = np.asarray(att_src, np.float32)
    att_dst = np.asarray(att_dst, np.float32)
    bias = np.asarray(bias, np.float32)

    wa = np.zeros((P, 136), np.float32)
    wa[:, :128] = W
    for hh in range(H):
        wa[:, 128 + hh] = W[:, hh * C:(hh + 1) * C] @ att_src[hh]
        # one-hot entries are 2.0 (bit shifted to fp16 exponent); halve a_dst
        wa[:, 132 + hh] = 0.5 * (W[:, hh * C:(hh + 1) * C] @ att_dst[hh])
    wa_bf = wa.astype(BF16)

    x_pad = np.zeros((NPAD, P), np.float32)
    x_pad[:N] = x
    x_bf = x_pad.astype(BF16)

    K, groups, idx_maps, bm_maps = _prep_edges(np.asarray(edge_index))
    plan, TOTC = _plan(K, groups)
    bmi_maps = _interleave_bitmaps(plan, TOTC, bm_maps)
    nc = _build_program(K, groups)

    bias_rep = np.tile(bias[None, :], (P, 1)).astype(np.float32)

    in_maps = []
    for c in range(NCORE):
        xc = np.roll(x_bf, -c * B, axis=0)
        in_maps.append({
            "xbf": np.ascontiguousarray(xc),
            "waug": wa_bf,
            "idx_all": idx_maps[c],
            "bm_all": bmi_maps[c],
            "bias_in": bias_rep,
        })
    return nc, in_maps


def kernel(x, edge_index, W, att_src, att_dst, bias):
    nc, in_maps = prepare(x, edge_index, W, att_src, att_dst, bias)
    res = run_bass_kernel_spmd(nc, in_maps, list(range(NCORE)))
    out = np.empty((NPAD, C), np.float32)
    for c in range(NCORE):
        out[c * B:(c + 1) * B] = res.results[c]["out"]
    return out[:N]


# revision 35
# speedup vs baseline: 2.3904x; 1.0233x over previous
"""GATConv forward on 8 Trainium2 NeuronCores (Bass/Tile).

Strategy: destination-node sharding. Host sorts edges by dst, assigns each
core a contiguous dst range (12544 nodes = 98 tiles of 128). Node ids are
cyclically renumbered per core so every core's local nodes are 0..12543 and
the SPMD program is identical across cores; all per-core variation lives in
the input data.

Phase A: per 128-node tile, [h | a_src | 0.5*a_dst] = x @ waug on the PE;
rows packed into a 512B-row HBM table T as [h bf16 (256B) | a_src fp16
(8B) | pad]; 0.5*a_dst for local tiles kept in SBUF (slot-partitioned,
fp16).

Phase B: dst tiles are bin-packed into groups of 3 and processed in a
software pipeline so each engine's in-order queue never head-of-line
blocks on another engine:
  iteration i issues  gather(i)+streams(i+1) | bitexpand(i) | a_dst(i)
  [PE] | z/w/msg(i-1) [DVE+Act] | scatter(i-1) [PE] | epilogue(i-2).
dma_gathers are capped at 8 chunks (1024-descriptor SWDGE ring limit).
One-hot matrices come from host-packed uint16 bitmaps bit-expanded with
(x & (1<<b)) << (14-b), whose u16 result 0x4000 bitcast to fp16 is exactly
2.0 (scale cancels in the softmax ratio; att_dst is pre-halved). a_dst per
edge = tiny PE matmuls s01T^T @ adst; w = exp(lrelu(a_src + a_dst));
messages [w*h | w] scatter into a per-group [128,3,132] PSUM accumulator;
epilogue relu(mean_h num/den + bias).
"""
import sys

sys.path.insert(0, "/opt/trn_rl_repo")
import numpy as np
import ml_dtypes

import concourse.bass as bass
import concourse.mybir as mybir
import concourse.tile as tile
from concourse.bass_utils import run_bass_kernel_spmd
from concourse import bacc

BF16 = ml_dtypes.bfloat16
P = 128
N = 100000
NPAD = 100352          # 784 tiles of 128; 8 cores x 12544
NCORE = 8
B = NPAD // NCORE      # 12544 local nodes per core
TPC = B // P           # 98 tiles per core
NTILE = NPAD // P      # 784 global row tiles
BANK = 32768
NBANK = (NPAD + BANK - 1) // BANK  # 4
NEG = 0.2
H, C = 4, 32
G = 3                  # dst tiles per group (bin-packed)
EW = 256               # T row width in u16 elements (512B)


def _prep_edges(edge_index):
    src0 = edge_index[0].astype(np.int64)
    dst0 = edge_index[1].astype(np.int64)
    loops = np.arange(NPAD, dtype=np.int64)
    src = np.concatenate([src0, loops])
    dst = np.concatenate([dst0, loops])

    per_core = []
    cnts = np.zeros((NCORE, TPC * NBANK), np.int64)
    for c in range(NCORE):
        lo, hi = c * B, (c + 1) * B
        sel = (dst >= lo) & (dst < hi)
        d = dst[sel] - lo
        s = (src[sel] - lo) % NPAD
        t = d >> 7
        sl = d & 127
        bk = s >> 15
        il = s & (BANK - 1)
        q = t * NBANK + bk
        per_core.append((q, il, sl))
        cnts[c] = np.bincount(q, minlength=TPC * NBANK)

    K = np.ceil(cnts.max(axis=0) / P).astype(np.int64).reshape(TPC, NBANK)

    # bin-pack tiles into groups of G, balancing total chunks per group
    w = K.sum(axis=1)
    order = np.argsort(-w, kind="stable")
    ngrp = (TPC + G - 1) // G
    bins = [[] for _ in range(ngrp)]
    load = np.zeros(ngrp, np.int64)
    for t in order:
        cand = [b for b in range(ngrp) if len(bins[b]) < G]
        b = min(cand, key=lambda i: load[i])
        bins[b].append(int(t))
        load[b] += w[t]
    groups = [sorted(b) for b in bins]

    # global chunk layout: (group, bank, tile, chunk)
    qorder = []
    for tiles in groups:
        for bk in range(NBANK):
            for t in tiles:
                qorder.append(t * NBANK + bk)
    qorder = np.array(qorder, np.int64)
    sz_by_q = (K.reshape(-1) * P)
    sz_in_order = sz_by_q[qorder]
    goff_in_order = np.zeros(len(qorder) + 1, np.int64)
    np.cumsum(sz_in_order, out=goff_in_order[1:])
    tot = int(goff_in_order[-1])
    qoff = np.zeros(TPC * NBANK, np.int64)
    qoff[qorder] = goff_in_order[:-1]

    TOTC = tot // P
    idx_maps, bm_maps = [], []
    for c in range(NCORE):
        q, il, sl = per_core[c]
        cnt = cnts[c]
        start = np.zeros(TPC * NBANK + 1, np.int64)
        np.cumsum(cnt, out=start[1:])
        order_e = np.argsort(q, kind="stable")
        qs = q[order_e]
        rank = np.arange(len(qs)) - start[qs]
        pos = qoff[qs] + rank               # global padded edge position
        idx_pad = np.zeros(tot, np.int16)
        idx_pad[pos] = il[order_e].astype(np.int16)

        # idx table: per chunk [16, 8] wrap -> [16, tot/16], replicated to 128
        idx16 = np.ascontiguousarray(
            idx_pad.reshape(TOTC, 8, 16).transpose(2, 0, 1).reshape(16, TOTC * 8))
        idx128 = np.ascontiguousarray(np.tile(idx16, (8, 1)))

        cc = pos >> 7                       # chunk of each real edge
        lane = pos & 127                    # partition lane within chunk
        slv = sl[order_e]                   # slot (dst & 127) of each edge
        # bmg[e, cc*8 + slot//16] bit slot%16  (edge-partitioned, bits=slot)
        bmg = np.zeros((P, TOTC * 8), np.uint16)
        np.bitwise_or.at(bmg, (lane, cc * 8 + (slv >> 4)),
                         (1 << (slv & 15)).astype(np.uint16))
        # bmt[slot, cc*8 + lane//16] bit lane%16 (slot-partitioned, bits=edge)
        bmt = np.zeros((P, TOTC * 8), np.uint16)
        np.bitwise_or.at(bmt, (slv, cc * 8 + (lane >> 4)),
                         (1 << (lane & 15)).astype(np.uint16))
        idx_maps.append(idx128)
        bm_maps.append((bmg, bmt))
    return K, groups, idx_maps, bm_maps


def _plan(K, groups):
    """Per-group program metadata (shared across cores)."""
    plan = []
    cc = 0
    for tiles in groups:
        gathers = []   # (bank, nch, chunk_off_in_group); nch <= 8
        off = 0
        for bk in range(NBANK):
            nch_bk = int(K[tiles, bk].sum())
            for p0 in range(0, nch_bk, 8):
                gathers.append((bk, min(8, nch_bk - p0), off + p0))
            off += nch_bk
        kg = off
        # chunk -> owning tile (local index), in (bank, tile, chunk) order
        owner = []
        for bk in range(NBANK):
            for ti, t in enumerate(tiles):
                owner += [ti] * int(K[t, bk])
        # scatter order: tile-major so accumulation groups don't interleave
        scatter = []   # (t_local, chunk idx, start, stop)
        for ti, t in enumerate(tiles):
            ks = [k for k in range(kg) if owner[k] == ti]
            for i, k in enumerate(ks):
                scatter.append((ti, k, i == 0, i == len(ks) - 1))
        plan.append(dict(tiles=tiles, gathers=gathers, kg=kg, cc0=cc,
                         owner=owner, scatter=scatter))
        cc += kg
    return plan, cc


def _interleave_bitmaps(plan, TOTC, bm_maps):
    out = []
    for bmg, bmt in bm_maps:
        m = np.zeros((P, TOTC * 16), np.uint16)
        for pl in plan:
            c0, kg = pl["cc0"], pl["kg"]
            m[:, c0 * 16:c0 * 16 + kg * 8] = bmg[:, c0 * 8:(c0 + kg) * 8]
            m[:, c0 * 16 + kg * 8:(c0 + kg) * 16] = bmt[:, c0 * 8:(c0 + kg) * 8]
        out.append(m)
    return out


def _build_program(K, groups):
    plan, TOTC = _plan(K, groups)
    NG = len(plan)
    TOT16 = TOTC * 8
    f32, bf16, fp16, fp8, i16, u16 = (
        mybir.dt.float32, mybir.dt.bfloat16, mybir.dt.float16,
        mybir.dt.float8e4, mybir.dt.int16, mybir.dt.uint16)
    AF = mybir.ActivationFunctionType
    OP = mybir.AluOpType

    nc = bacc.Bacc("TRN2", target_bir_lowering=False, debug=False,
                   num_devices=NCORE)
    xbf = nc.dram_tensor("xbf", [NPAD, P], bf16, kind="ExternalInput")
    waug = nc.dram_tensor("waug", [P, 136], bf16, kind="ExternalInput")
    idx_all = nc.dram_tensor("idx_all", [P, TOT16], i16, kind="ExternalInput")
    bm_all = nc.dram_tensor("bm_all", [P, TOTC * 16], u16, kind="ExternalInput")
    bias_in = nc.dram_tensor("bias_in", [P, C], f32, kind="ExternalInput")
    T = nc.dram_tensor("T", [NPAD, EW], u16)
    out_d = nc.dram_tensor("out", [B, C], f32, kind="ExternalOutput")

    Tv = T[:, :].rearrange("(t p) e -> p t e", p=P)         # [128, 784, 128]
    out_v = out_d[:, :].rearrange("(t p) c -> p t c", p=P)  # [128, 98, 32]

    with tile.TileContext(nc) as tc:
        with tc.tile_pool(name="const", bufs=1) as cp:
            waug_sb = cp.tile([P, 136], bf16)
            nc.sync.dma_start(out=waug_sb[:], in_=waug[:, :])
            bias_sb = cp.tile([P, C], f32)
            nc.sync.dma_start(out=bias_sb[:], in_=bias_in[:, :])
            adst_sb = cp.tile([P, TPC, 4], fp16)
            outall_sb = cp.tile([P, TPC, C], f32)

            # ------------- Phase A: T = x @ waug, a_dst table --------------
            # Chunked into few, large DMAs: the tile scheduler serializes
            # any DMA against the next one with a ~5.5us bubble, so one
            # 8192-row transpose-in and one 64-tile T-write-out per chunk
            # bounds that cost to ~13 bubbles total. T writes go through
            # HWDGE (Act queue) -- SWDGE's 1024-descriptor ring cannot take
            # an 8192-descriptor DMA.
            CH = 16384
            NCHUNK = (NPAD + CH - 1) // CH  # 7 (last chunk 2048 rows)
            with tc.tile_pool(name="pax", bufs=2) as pax, \
                 tc.tile_pool(name="pat", bufs=2) as pat, \
                 tc.tile_pool(name="psa", bufs=2, space="PSUM") as psa:
                for ck in range(NCHUNK):
                    r0 = ck * CH
                    nrows = min(CH, NPAD - r0)
                    nt = nrows // P                  # tiles in chunk (<=64)
                    xT = pax.tile([P, CH], bf16, tag="xT")
                    nc.sync.dma_start(out=xT[:, 0:nrows],
                                      in_=xbf[r0:r0 + nrows, :],
                                      transpose=True)
                    Tb = pat.tile([P, CH // P, 132], u16, tag="Tb")
                    for q in range(nt // 4):
                        ps = psa.tile([P, 4, 512], f32, tag="psA",
                                      space="PSUM")
                        for i in range(4):
                            nc.tensor.matmul(
                                out=ps[:, i, 0:136],
                                lhsT=xT[:, (q * 4 + i) * P:(q * 4 + i + 1) * P],
                                rhs=waug_sb[:], start=True, stop=True)
                        q4 = slice(q * 4, (q + 1) * 4)
                        if q % 2 == 0:
                            nc.scalar.activation(
                                out=Tb[:, q4, 0:128].bitcast(bf16),
                                in_=ps[:, :, 0:128], func=AF.Copy)
                        else:
                            nc.vector.tensor_copy(
                                out=Tb[:, q4, 0:128].bitcast(bf16),
                                in_=ps[:, :, 0:128])
                        nc.scalar.activation(
                            out=Tb[:, q4, 128:132].bitcast(fp16),
                            in_=ps[:, :, 128:132], func=AF.Copy)
                        t0 = r0 // P + q * 4
                        if t0 < TPC:
                            nloc = min(4, TPC - t0)
                            nc.scalar.activation(
                                out=adst_sb[:, t0:t0 + nloc, :],
                                in_=ps[:, 0:nloc, 132:136], func=AF.Copy)
                    nc.scalar.dma_start(
                        out=Tv[:, r0 // P:r0 // P + nt, 0:132],
                        in_=Tb[:, 0:nt, :])

            tc.strict_bb_all_engine_barrier()

            # ------------- Phase B: software-pipelined groups --------------
            with tc.tile_pool(name="pgt", bufs=2) as pgt, \
                 tc.tile_pool(name="pix", bufs=2) as pix, \
                 tc.tile_pool(name="pbm", bufs=2) as pbm, \
                 tc.tile_pool(name="ptr", bufs=2) as ptr, \
                 tc.tile_pool(name="pt1", bufs=1) as pt1, \
                 tc.tile_pool(name="psm", bufs=2) as psm, \
                 tc.tile_pool(name="pms", bufs=1) as pms, \
                 tc.tile_pool(name="psb", bufs=3, space="PSUM") as psb, \
                 tc.tile_pool(name="psd", bufs=2, space="PSUM") as psd:
                st = {}  # live per-group tiles
                nregs = {}
                for pl_ in plan:
                    for _, nch_, _ in pl_["gathers"]:
                        nregs.setdefault(nch_ * P, None)
                for v in sorted(nregs):
                    nregs[v] = nc.gpsimd.to_reg(v)

                def issue_streams(g):
                    pl = plan[g]
                    kg, cc0 = pl["kg"], pl["cc0"]
                    idx_t = pix.tile([P, kg * 8], i16, tag="idx")
                    nc.scalar.dma_start(
                        out=idx_t[:], in_=idx_all[:, cc0 * 8:(cc0 + kg) * 8])
                    bm_t = pbm.tile([P, 2, kg * 8], u16, tag="bm")
                    nc.scalar.dma_start(
                        out=bm_t[:].rearrange("p a b -> p (a b)"),
                        in_=bm_all[:, cc0 * 16:(cc0 + kg) * 16])
                    st[g] = {"idx": idx_t, "bm": bm_t}

                def issue_gather(g):
                    pl = plan[g]
                    kg = pl["kg"]
                    gt = pgt.tile([P, kg, EW], u16, tag="gath")
                    idx_t = st[g]["idx"]
                    for bk, nch, off in pl["gathers"]:
                        rows = min(BANK, NPAD - bk * BANK)
                        nc.gpsimd.dma_gather(
                            out_ap=gt[:, off:off + nch, :],
                            in_ap=T[bk * BANK:bk * BANK + rows, :],
                            idxs_ap=idx_t[:, off * 8:(off + nch) * 8],
                            num_idxs=nch * P, num_idxs_reg=nregs[nch * P],
                            elem_size=EW)
                    st[g]["gt"] = gt

                def issue_bitexp(g):
                    pl = plan[g]
                    kg = pl["kg"]
                    k8 = kg * 8
                    bm_t = st[g]["bm"]
                    bxg = ptr.tile([P, k8, 16], u16, tag="bxg")
                    bxt = pt1.tile([P, k8, 16], u16, tag="bxt")
                    for b in range(16):
                        sh = (OP.logical_shift_left if b <= 14
                              else OP.logical_shift_right)
                        nc.vector.tensor_scalar(
                            out=bxt[:, :, b],
                            in0=bm_t[:, 1], scalar1=1 << b, scalar2=abs(14 - b),
                            op0=OP.bitwise_and, op1=sh)
                        nc.vector.tensor_scalar(
                            out=bxg[:, :, b],
                            in0=bm_t[:, 0], scalar1=1 << b, scalar2=abs(14 - b),
                            op0=OP.bitwise_and, op1=sh)
                    st[g]["s01g"] = bxg[:].bitcast(fp16).rearrange(
                        "p (k w) b -> p k (w b)", w=8)
                    st[g]["s01t"] = bxt[:].bitcast(fp16).rearrange(
                        "p (k w) b -> p k (w b)", w=8)

                def issue_adt(g):
                    pl = plan[g]
                    kg, tiles = pl["kg"], pl["tiles"]
                    s01t = st[g]["s01t"]
                    adt_ps = psd.tile([P, kg, 4], f32, tag="adt", space="PSUM")
                    for k in range(kg):
                        nc.tensor.matmul(
                            out=adt_ps[:, k, :], lhsT=s01t[:, k, :],
                            rhs=adst_sb[:, tiles[pl["owner"][k]], :],
                            start=True, stop=True)
                    st[g]["adt"] = adt_ps

                def issue_z(g):
                    pl = plan[g]
                    kg = pl["kg"]
                    gt = st[g]["gt"]
                    aview = gt[:, :, 128:132].bitcast(fp16)  # [P, kg, 4]
                    zt = psm.tile([P, kg, 4], fp16, tag="zt")
                    nc.vector.tensor_tensor(out=zt[:], in0=aview,
                                            in1=st[g]["adt"][:], op=OP.add)
                    lr = psm.tile([P, kg * 4], fp16, tag="lr")
                    nc.vector.scalar_tensor_tensor(
                        out=lr[:], in0=zt[:].rearrange("p k f -> p (k f)"),
                        scalar=NEG, in1=zt[:].rearrange("p k f -> p (k f)"),
                        op0=OP.mult, op1=OP.max)
                    wb = psm.tile([P, kg, 4], fp16, tag="wb")
                    nc.scalar.activation(
                        out=wb[:].rearrange("p k f -> p (k f)"), in_=lr[:],
                        func=AF.Exp)
                    wbx = pms.tile([P, kg, H, C], fp16, tag="wbx")
                    nc.scalar.activation(
                        out=wbx[:],
                        in_=wb[:, :, :, None].to_broadcast([P, kg, H, C]),
                        func=AF.Copy)
                    st[g]["wb"] = wb
                    st[g]["wbx"] = wbx

                def issue_msg2(g):
                    pl = plan[g]
                    kg = pl["kg"]
                    gt = st[g]["gt"]
                    hview = gt[:, :, 0:128].bitcast(bf16)    # [P, kg, 128]
                    msg = pms.tile([P, kg, 132], fp16, tag="msg")
                    nc.vector.tensor_tensor(
                        out=msg[:, :, 0:128],
                        in0=hview,
                        in1=st[g]["wbx"][:].rearrange("p k h c -> p (k h c)")
                            .rearrange("p (k f) -> p k f", k=kg),
                        op=OP.mult)
                    nc.scalar.activation(out=msg[:, :, 128:132],
                                         in_=st[g]["wb"][:], func=AF.Copy)
                    st[g]["msg"] = msg

                def issue_scatter(g):
                    pl = plan[g]
                    msg, s01g = st[g]["msg"], st[g]["s01g"]
                    acc = psb.tile([P, G, 132], f32, tag="acc", space="PSUM")
                    for ti, k, first, last in pl["scatter"]:
                        nc.tensor.matmul(out=acc[:, ti, :], lhsT=s01g[:, k, :],
                                         rhs=msg[:, k, 0:132],
                                         start=first, stop=last)
                    st[g]["acc"] = acc

                def issue_epilogue(g):
                    pl = plan[g]
                    tiles = pl["tiles"]
                    ng = len(tiles)
                    acc = st[g]["acc"]
                    den = psm.tile([P, G, 4], f32, tag="den")
                    nc.vector.reciprocal(out=den[:, 0:ng, :],
                                         in_=acc[:, 0:ng, 128:132])
                    tmp = psm.tile([P, G, P], f32, tag="tmp")
                    nc.vector.tensor_tensor(
                        out=tmp[:, 0:ng, :].rearrange("p g (h c) -> p g h c", h=H),
                        in0=acc[:, 0:ng, 0:128].rearrange("p g (h c) -> p g h c", h=H),
                        in1=den[:, 0:ng, :, None].to_broadcast([P, ng, H, C]),
                        op=OP.mult)
                    hsum = psm.tile([P, G, C], f32, tag="hsum")
                    nc.vector.tensor_reduce(
                        out=hsum[:, 0:ng, :],
                        in_=tmp[:, 0:ng, :].rearrange("p g (h c) -> p g c h", h=H),
                        axis=mybir.AxisListType.X, op=OP.add)
                    badd = psm.tile([P, G, C], f32, tag="badd")
                    nc.vector.scalar_tensor_tensor(
                        out=badd[:, 0:ng, :], in0=hsum[:, 0:ng, :],
                        scalar=1.0 / H,
                        in1=bias_sb[:, None, :].to_broadcast([P, ng, C]),
                        op0=OP.mult, op1=OP.add)
                    for ti, t in enumerate(tiles):
                        nc.vector.tensor_scalar_max(
                            out=outall_sb[:, t, :], in0=badd[:, ti, :],
                            scalar1=0.0)
                    del st[g]

                issue_streams(0)
                for i in range(NG + 2):
                    if 1 <= i <= NG:
                        issue_z(i - 1)
                    if i + 1 < NG:
                        issue_streams(i + 1)
                    if i < NG:
                        issue_gather(i)
                        issue_bitexp(i)
                        issue_adt(i)
                    if 1 <= i <= NG:
                        issue_msg2(i - 1)
                        issue_scatter(i - 1)
                    if i >= 2:
                        issue_epilogue(i - 2)
                nc.sync.dma_start(out=out_v[:, :, :], in_=outall_sb[:])
    nc.compile()
    return nc


def prepare(x, edge_index, W, att_src, att_dst, bias):
    x = np.asarray(x, np.float32)
    W = np.asarray(W, np.float32)
    att_src = np.asarray(att_src, np.float32)
    att_dst = np.asarray(att_dst, np.float32)
    bias = np.asarray(bias, np.float32)

    wa = np.zeros((P, 136), np.float32)
    wa[:, :128] = W
    for hh in range(H):
        wa[:, 128 + hh] = W[:, hh * C:(hh + 1) * C] @ att_src[hh]
        # one-hot entries are 2.0 (bit shifted to fp16 exponent); halve a_dst
        wa[:, 132 + hh] = 0.5 * (W[:, hh * C:(hh + 1) * C] @ att_dst[hh])
    wa_bf = wa.astype(BF16)

    x_pad = np.zeros((NPAD, P), np.float32)
    x_pad[:N] = x
    x_bf = x_pad.astype(BF16)

    K, groups, idx_maps, bm_maps = _prep_edges(np.asarray(edge_index))
    plan, TOTC = _plan(K, groups)
    bmi_maps = _interleave_bitmaps(plan, TOTC, bm_maps)
    nc = _build_program(K, groups)

    bias_rep = np.tile(bias[None, :], (P, 1)).astype(np.float32)

    in_maps = []
    for c in range(NCORE):
        xc = np.roll(x_bf, -c * B, axis=0)
        in_maps.append({
            "xbf": np.ascontiguousarray(xc),
            "waug": wa_bf,
            "idx_all": idx_maps[c],
            "bm_all": bmi_maps[c],
            "bias_in": bias_rep,
        })
    return nc, in_maps


def kernel(x, edge_index, W, att_src, att_dst, bias):
    nc, in_maps = prepare(x, edge_index, W, att_src, att_dst, bias)
    res = run_bass_kernel_spmd(nc, in_maps, list(range(NCORE)))
    out = np.empty((NPAD, C), np.float32)
    for c in range(NCORE):
        out[c * B:(c + 1) * B] = res.results[c]["out"]
    return out[:N]


# revision 36
# speedup vs baseline: 2.5395x; 1.0624x over previous
"""GATConv forward on 8 Trainium2 NeuronCores (Bass/Tile).

Strategy: destination-node sharding. Host sorts edges by dst, assigns each
core a contiguous dst range (12544 nodes = 98 tiles of 128). Node ids are
cyclically renumbered per core so every core's local nodes are 0..12543 and
the SPMD program is identical across cores; all per-core variation lives in
the input data.

Phase A: per 128-node tile, [h | a_src | 0.5*a_dst] = x @ waug on the PE;
rows packed into a 512B-row HBM table T as [h bf16 (256B) | a_src fp16
(8B) | pad]; 0.5*a_dst for local tiles kept in SBUF (slot-partitioned,
fp16).

Phase B: dst tiles are bin-packed into groups of 3 and processed in a
software pipeline so each engine's in-order queue never head-of-line
blocks on another engine:
  iteration i issues  gather(i)+streams(i+1) | bitexpand(i) | a_dst(i)
  [PE] | z/w/msg(i-1) [DVE+Act] | scatter(i-1) [PE] | epilogue(i-2).
dma_gathers are capped at 8 chunks (1024-descriptor SWDGE ring limit).
One-hot matrices come from host-packed uint16 bitmaps bit-expanded with
(x & (1<<b)) << (14-b), whose u16 result 0x4000 bitcast to fp16 is exactly
2.0 (scale cancels in the softmax ratio; att_dst is pre-halved). a_dst per
edge = tiny PE matmuls s01T^T @ adst; w = exp(lrelu(a_src + a_dst));
messages [w*h | w] scatter into a per-group [128,3,132] PSUM accumulator;
epilogue relu(mean_h num/den + bias).
"""
import sys

sys.path.insert(0, "/opt/trn_rl_repo")
import numpy as np
import ml_dtypes

import concourse.bass as bass
import concourse.mybir as mybir
import concourse.tile as tile
from concourse.bass_utils import run_bass_kernel_spmd
from concourse import bacc

BF16 = ml_dtypes.bfloat16
P = 128
N = 100000
NPAD = 100352          # 784 tiles of 128; 8 cores x 12544
NCORE = 8
B = NPAD // NCORE      # 12544 local nodes per core
TPC = B // P           # 98 tiles per core
NTILE = NPAD // P      # 784 global row tiles
BANK = 32768
NBANK = (NPAD + BANK - 1) // BANK  # 4
NEG = 0.2
H, C = 4, 32
G = 3                  # dst tiles per group (bin-packed)
EW = 256               # T row width in u16 elements (512B)


def _prep_edges(edge_index):
    src0 = edge_index[0].astype(np.int64)
    dst0 = edge_index[1].astype(np.int64)
    loops = np.arange(NPAD, dtype=np.int64)
    src = np.concatenate([src0, loops])
    dst = np.concatenate([dst0, loops])

    per_core = []
    cnts = np.zeros((NCORE, TPC * NBANK), np.int64)
    for c in range(NCORE):
        lo, hi = c * B, (c + 1) * B
        sel = (dst >= lo) & (dst < hi)
        d = dst[sel] - lo
        s = (src[sel] - lo) % NPAD
        t = d >> 7
        sl = d & 127
        bk = s >> 15
        il = s & (BANK - 1)
        q = t * NBANK + bk
        per_core.append((q, il, sl))
        cnts[c] = np.bincount(q, minlength=TPC * NBANK)

    K = np.ceil(cnts.max(axis=0) / P).astype(np.int64).reshape(TPC, NBANK)

    # bin-pack tiles into groups of G, balancing total chunks per group
    w = K.sum(axis=1)
    order = np.argsort(-w, kind="stable")
    ngrp = (TPC + G - 1) // G
    bins = [[] for _ in range(ngrp)]
    load = np.zeros(ngrp, np.int64)
    for t in order:
        cand = [b for b in range(ngrp) if len(bins[b]) < G]
        b = min(cand, key=lambda i: load[i])
        bins[b].append(int(t))
        load[b] += w[t]
    groups = [sorted(b) for b in bins]

    # global chunk layout: (group, bank, tile, chunk)
    qorder = []
    for tiles in groups:
        for bk in range(NBANK):
            for t in tiles:
                qorder.append(t * NBANK + bk)
    qorder = np.array(qorder, np.int64)
    sz_by_q = (K.reshape(-1) * P)
    sz_in_order = sz_by_q[qorder]
    goff_in_order = np.zeros(len(qorder) + 1, np.int64)
    np.cumsum(sz_in_order, out=goff_in_order[1:])
    tot = int(goff_in_order[-1])
    qoff = np.zeros(TPC * NBANK, np.int64)
    qoff[qorder] = goff_in_order[:-1]

    TOTC = tot // P
    idx_maps, bm_maps = [], []
    for c in range(NCORE):
        q, il, sl = per_core[c]
        cnt = cnts[c]
        start = np.zeros(TPC * NBANK + 1, np.int64)
        np.cumsum(cnt, out=start[1:])
        order_e = np.argsort(q, kind="stable")
        qs = q[order_e]
        rank = np.arange(len(qs)) - start[qs]
        pos = qoff[qs] + rank               # global padded edge position
        idx_pad = np.zeros(tot, np.int16)
        idx_pad[pos] = il[order_e].astype(np.int16)

        # idx table: per chunk [16, 8] wrap -> [16, tot/16], replicated to 128
        idx16 = np.ascontiguousarray(
            idx_pad.reshape(TOTC, 8, 16).transpose(2, 0, 1).reshape(16, TOTC * 8))
        idx128 = np.ascontiguousarray(np.tile(idx16, (8, 1)))

        cc = pos >> 7                       # chunk of each real edge
        lane = pos & 127                    # partition lane within chunk
        slv = sl[order_e]                   # slot (dst & 127) of each edge
        # bmg[e, cc*8 + slot//16] bit slot%16  (edge-partitioned, bits=slot)
        bmg = np.zeros((P, TOTC * 8), np.uint16)
        np.bitwise_or.at(bmg, (lane, cc * 8 + (slv >> 4)),
                         (1 << (slv & 15)).astype(np.uint16))
        # bmt[slot, cc*8 + lane//16] bit lane%16 (slot-partitioned, bits=edge)
        bmt = np.zeros((P, TOTC * 8), np.uint16)
        np.bitwise_or.at(bmt, (slv, cc * 8 + (lane >> 4)),
                         (1 << (lane & 15)).astype(np.uint16))
        idx_maps.append(idx128)
        bm_maps.append((bmg, bmt))
    return K, groups, idx_maps, bm_maps


def _plan(K, groups):
    """Per-group program metadata (shared across cores)."""
    plan = []
    cc = 0
    for tiles in groups:
        gathers = []   # (bank, nch, chunk_off_in_group); nch <= 8
        off = 0
        for bk in range(NBANK):
            nch_bk = int(K[tiles, bk].sum())
            for p0 in range(0, nch_bk, 8):
                gathers.append((bk, min(8, nch_bk - p0), off + p0))
            off += nch_bk
        kg = off
        # chunk -> owning tile (local index), in (bank, tile, chunk) order
        owner = []
        for bk in range(NBANK):
            for ti, t in enumerate(tiles):
                owner += [ti] * int(K[t, bk])
        # scatter order: tile-major so accumulation groups don't interleave
        scatter = []   # (t_local, chunk idx, start, stop)
        for ti, t in enumerate(tiles):
            ks = [k for k in range(kg) if owner[k] == ti]
            for i, k in enumerate(ks):
                scatter.append((ti, k, i == 0, i == len(ks) - 1))
        plan.append(dict(tiles=tiles, gathers=gathers, kg=kg, cc0=cc,
                         owner=owner, scatter=scatter))
        cc += kg
    return plan, cc


def _interleave_bitmaps(plan, TOTC, bm_maps):
    out = []
    for bmg, bmt in bm_maps:
        m = np.zeros((P, TOTC * 16), np.uint16)
        for pl in plan:
            c0, kg = pl["cc0"], pl["kg"]
            m[:, c0 * 16:c0 * 16 + kg * 8] = bmg[:, c0 * 8:(c0 + kg) * 8]
            m[:, c0 * 16 + kg * 8:(c0 + kg) * 16] = bmt[:, c0 * 8:(c0 + kg) * 8]
        out.append(m)
    return out


def _build_program(K, groups):
    plan, TOTC = _plan(K, groups)
    NG = len(plan)
    TOT16 = TOTC * 8
    f32, bf16, fp16, fp8, i16, u16 = (
        mybir.dt.float32, mybir.dt.bfloat16, mybir.dt.float16,
        mybir.dt.float8e4, mybir.dt.int16, mybir.dt.uint16)
    AF = mybir.ActivationFunctionType
    OP = mybir.AluOpType

    nc = bacc.Bacc("TRN2", target_bir_lowering=False, debug=False,
                   num_devices=NCORE)
    xbf = nc.dram_tensor("xbf", [NPAD, P], bf16, kind="ExternalInput")
    waug = nc.dram_tensor("waug", [P, 136], bf16, kind="ExternalInput")
    idx_all = nc.dram_tensor("idx_all", [P, TOT16], i16, kind="ExternalInput")
    bm_all = nc.dram_tensor("bm_all", [P, TOTC * 16], u16, kind="ExternalInput")
    bias_in = nc.dram_tensor("bias_in", [P, C], f32, kind="ExternalInput")
    T = nc.dram_tensor("T", [NPAD, EW], u16)
    out_d = nc.dram_tensor("out", [B, C], f32, kind="ExternalOutput")

    Tv = T[:, :].rearrange("(t p) e -> p t e", p=P)         # [128, 784, 128]
    out_v = out_d[:, :].rearrange("(t p) c -> p t c", p=P)  # [128, 98, 32]

    with tile.TileContext(nc) as tc:
        with tc.tile_pool(name="const", bufs=1) as cp:
            waug_sb = cp.tile([P, 136], bf16)
            nc.sync.dma_start(out=waug_sb[:], in_=waug[:, :])
            bias_sb = cp.tile([P, C], f32)
            nc.sync.dma_start(out=bias_sb[:], in_=bias_in[:, :])
            adst_sb = cp.tile([P, TPC, 4], fp16)
            outall_sb = cp.tile([P, TPC, C], f32)

            # ------------- Phase A: T = x @ waug, a_dst table --------------
            # Chunked into few, large DMAs: the tile scheduler serializes
            # any DMA against the next one with a ~5.5us bubble, so one
            # 8192-row transpose-in and one 64-tile T-write-out per chunk
            # bounds that cost to ~13 bubbles total. T writes go through
            # HWDGE (Act queue) -- SWDGE's 1024-descriptor ring cannot take
            # an 8192-descriptor DMA.
            CH = 16384
            NCHUNK = (NPAD + CH - 1) // CH  # 7 (last chunk 2048 rows)
            with tc.tile_pool(name="pax", bufs=2) as pax, \
                 tc.tile_pool(name="pat", bufs=2) as pat, \
                 tc.tile_pool(name="psa", bufs=2, space="PSUM") as psa:
                for ck in range(NCHUNK):
                    r0 = ck * CH
                    nrows = min(CH, NPAD - r0)
                    nt = nrows // P                  # tiles in chunk (<=64)
                    xT = pax.tile([P, CH], bf16, tag="xT")
                    nc.sync.dma_start(out=xT[:, 0:nrows],
                                      in_=xbf[r0:r0 + nrows, :],
                                      transpose=True)
                    Tb = pat.tile([P, CH // P, 132], u16, tag="Tb")
                    for q in range(nt // 4):
                        ps = psa.tile([P, 4, 512], f32, tag="psA",
                                      space="PSUM")
                        for i in range(4):
                            nc.tensor.matmul(
                                out=ps[:, i, 0:136],
                                lhsT=xT[:, (q * 4 + i) * P:(q * 4 + i + 1) * P],
                                rhs=waug_sb[:], start=True, stop=True)
                        q4 = slice(q * 4, (q + 1) * 4)
                        if q % 2 == 0:
                            nc.scalar.activation(
                                out=Tb[:, q4, 0:128].bitcast(bf16),
                                in_=ps[:, :, 0:128], func=AF.Copy)
                        else:
                            nc.vector.tensor_copy(
                                out=Tb[:, q4, 0:128].bitcast(bf16),
                                in_=ps[:, :, 0:128])
                        nc.scalar.activation(
                            out=Tb[:, q4, 128:132].bitcast(fp16),
                            in_=ps[:, :, 128:132], func=AF.Copy)
                        t0 = r0 // P + q * 4
                        if t0 < TPC:
                            nloc = min(4, TPC - t0)
                            nc.scalar.activation(
                                out=adst_sb[:, t0:t0 + nloc, :],
                                in_=ps[:, 0:nloc, 132:136], func=AF.Copy)
                    nc.scalar.dma_start(
                        out=Tv[:, r0 // P:r0 // P + nt, 0:132],
                        in_=Tb[:, 0:nt, :])

            tc.strict_bb_all_engine_barrier()

            # ------------- Phase B: software-pipelined groups --------------
            with tc.tile_pool(name="pgt", bufs=2) as pgt, \
                 tc.tile_pool(name="pix", bufs=2) as pix, \
                 tc.tile_pool(name="pbm", bufs=2) as pbm, \
                 tc.tile_pool(name="ptr", bufs=3) as ptr, \
                 tc.tile_pool(name="pt1", bufs=1) as pt1, \
                 tc.tile_pool(name="psm", bufs=2) as psm, \
                 tc.tile_pool(name="pms", bufs=1) as pms, \
                 tc.tile_pool(name="psb", bufs=3, space="PSUM") as psb, \
                 tc.tile_pool(name="psd", bufs=3, space="PSUM") as psd:
                st = {}  # live per-group tiles
                nregs = {}
                for pl_ in plan:
                    for _, nch_, _ in pl_["gathers"]:
                        nregs.setdefault(nch_ * P, None)
                for v in sorted(nregs):
                    nregs[v] = nc.gpsimd.to_reg(v)

                def issue_streams(g):
                    pl = plan[g]
                    kg, cc0 = pl["kg"], pl["cc0"]
                    idx_t = pix.tile([P, kg * 8], i16, tag="idx")
                    nc.scalar.dma_start(
                        out=idx_t[:], in_=idx_all[:, cc0 * 8:(cc0 + kg) * 8])
                    bm_t = pbm.tile([P, 2, kg * 8], u16, tag="bm")
                    nc.scalar.dma_start(
                        out=bm_t[:].rearrange("p a b -> p (a b)"),
                        in_=bm_all[:, cc0 * 16:(cc0 + kg) * 16])
                    st[g] = {"idx": idx_t, "bm": bm_t}

                def issue_gather(g):
                    pl = plan[g]
                    kg = pl["kg"]
                    gt = pgt.tile([P, kg, EW], u16, tag="gath")
                    idx_t = st[g]["idx"]
                    for bk, nch, off in pl["gathers"]:
                        rows = min(BANK, NPAD - bk * BANK)
                        nc.gpsimd.dma_gather(
                            out_ap=gt[:, off:off + nch, :],
                            in_ap=T[bk * BANK:bk * BANK + rows, :],
                            idxs_ap=idx_t[:, off * 8:(off + nch) * 8],
                            num_idxs=nch * P, num_idxs_reg=nregs[nch * P],
                            elem_size=EW)
                    st[g]["gt"] = gt

                def issue_bitexp(g):
                    pl = plan[g]
                    kg = pl["kg"]
                    k8 = kg * 8
                    bm_t = st[g]["bm"]
                    bxg = ptr.tile([P, k8, 16], u16, tag="bxg")
                    bxt = pt1.tile([P, k8, 16], u16, tag="bxt")
                    for b in range(16):
                        sh = (OP.logical_shift_left if b <= 14
                              else OP.logical_shift_right)
                        nc.vector.tensor_scalar(
                            out=bxt[:, :, b],
                            in0=bm_t[:, 1], scalar1=1 << b, scalar2=abs(14 - b),
                            op0=OP.bitwise_and, op1=sh)
                        nc.vector.tensor_scalar(
                            out=bxg[:, :, b],
                            in0=bm_t[:, 0], scalar1=1 << b, scalar2=abs(14 - b),
                            op0=OP.bitwise_and, op1=sh)
                    st[g]["s01g"] = bxg[:].bitcast(fp16).rearrange(
                        "p (k w) b -> p k (w b)", w=8)
                    st[g]["s01t"] = bxt[:].bitcast(fp16).rearrange(
                        "p (k w) b -> p k (w b)", w=8)

                def issue_adt(g):
                    pl = plan[g]
                    kg, tiles = pl["kg"], pl["tiles"]
                    s01t = st[g]["s01t"]
                    adt_ps = psd.tile([P, kg, 4], f32, tag="adt", space="PSUM")
                    for k in range(kg):
                        nc.tensor.matmul(
                            out=adt_ps[:, k, :], lhsT=s01t[:, k, :],
                            rhs=adst_sb[:, tiles[pl["owner"][k]], :],
                            start=True, stop=True)
                    st[g]["adt"] = adt_ps

                def issue_z(g):
                    pl = plan[g]
                    kg = pl["kg"]
                    gt = st[g]["gt"]
                    aview = gt[:, :, 128:132].bitcast(fp16)  # [P, kg, 4]
                    zt = psm.tile([P, kg, 4], fp16, tag="zt")
                    nc.vector.tensor_tensor(out=zt[:], in0=aview,
                                            in1=st[g]["adt"][:], op=OP.add)
                    lr = psm.tile([P, kg * 4], fp16, tag="lr")
                    nc.vector.scalar_tensor_tensor(
                        out=lr[:], in0=zt[:].rearrange("p k f -> p (k f)"),
                        scalar=NEG, in1=zt[:].rearrange("p k f -> p (k f)"),
                        op0=OP.mult, op1=OP.max)
                    wb = psm.tile([P, kg, 4], fp16, tag="wb")
                    nc.scalar.activation(
                        out=wb[:].rearrange("p k f -> p (k f)"), in_=lr[:],
                        func=AF.Exp)
                    wbx = pms.tile([P, kg, H, C], fp16, tag="wbx")
                    nc.scalar.activation(
                        out=wbx[:],
                        in_=wb[:, :, :, None].to_broadcast([P, kg, H, C]),
                        func=AF.Copy)
                    st[g]["wb"] = wb
                    st[g]["wbx"] = wbx

                def issue_msg2(g):
                    pl = plan[g]
                    kg = pl["kg"]
                    gt = st[g]["gt"]
                    hview = gt[:, :, 0:128].bitcast(bf16)    # [P, kg, 128]
                    msg = pms.tile([P, kg, 132], fp16, tag="msg")
                    nc.vector.tensor_tensor(
                        out=msg[:, :, 0:128],
                        in0=hview,
                        in1=st[g]["wbx"][:].rearrange("p k h c -> p (k h c)")
                            .rearrange("p (k f) -> p k f", k=kg),
                        op=OP.mult)
                    nc.scalar.activation(out=msg[:, :, 128:132],
                                         in_=st[g]["wb"][:], func=AF.Copy)
                    st[g]["msg"] = msg

                def issue_scatter(g):
                    pl = plan[g]
                    msg, s01g = st[g]["msg"], st[g]["s01g"]
                    acc = psb.tile([P, G, 132], f32, tag="acc", space="PSUM")
                    for ti, k, first, last in pl["scatter"]:
                        nc.tensor.matmul(out=acc[:, ti, :], lhsT=s01g[:, k, :],
                                         rhs=msg[:, k, 0:132],
                                         start=first, stop=last)
                    st[g]["acc"] = acc

                def issue_epilogue(g):
                    pl = plan[g]
                    tiles = pl["tiles"]
                    ng = len(tiles)
                    acc = st[g]["acc"]
                    den = psm.tile([P, G, 4], f32, tag="den")
                    nc.vector.reciprocal(out=den[:, 0:ng, :],
                                         in_=acc[:, 0:ng, 128:132])
                    tmp = psm.tile([P, G, P], f32, tag="tmp")
                    nc.vector.tensor_tensor(
                        out=tmp[:, 0:ng, :].rearrange("p g (h c) -> p g h c", h=H),
                        in0=acc[:, 0:ng, 0:128].rearrange("p g (h c) -> p g h c", h=H),
                        in1=den[:, 0:ng, :, None].to_broadcast([P, ng, H, C]),
                        op=OP.mult)
                    hsum = psm.tile([P, G, C], f32, tag="hsum")
                    nc.vector.tensor_reduce(
                        out=hsum[:, 0:ng, :],
                        in_=tmp[:, 0:ng, :].rearrange("p g (h c) -> p g c h", h=H),
                        axis=mybir.AxisListType.X, op=OP.add)
                    badd = psm.tile([P, G, C], f32, tag="badd")
                    nc.vector.scalar_tensor_tensor(
                        out=badd[:, 0:ng, :], in0=hsum[:, 0:ng, :],
                        scalar=1.0 / H,
                        in1=bias_sb[:, None, :].to_broadcast([P, ng, C]),
                        op0=OP.mult, op1=OP.add)
                    for ti, t in enumerate(tiles):
                        nc.vector.tensor_scalar_max(
                            out=outall_sb[:, t, :], in0=badd[:, ti, :],
                            scalar1=0.0)
                    del st[g]

                issue_streams(0)
                for i in range(NG + 2):
                    if 1 <= i <= NG:
                        issue_z(i - 1)
                    if i + 1 < NG:
                        issue_streams(i + 1)
                    if i < NG:
                        issue_gather(i)
                        issue_bitexp(i)
                        issue_adt(i)
                    if 1 <= i <= NG:
                        issue_msg2(i - 1)
                        issue_scatter(i - 1)
                    if i >= 2:
                        issue_epilogue(i - 2)
                nc.sync.dma_start(out=out_v[:, :, :], in_=outall_sb[:])
    nc.compile()
    return nc


def prepare(x, edge_index, W, att_src, att_dst, bias):
    x = np.asarray(x, np.float32)
    W = np.asarray(W, np.float32)
    att_src = np.asarray(att_src, np.float32)
    att_dst = np.asarray(att_dst, np.float32)
    bias = np.asarray(bias, np.float32)

    wa = np.zeros((P, 136), np.float32)
    wa[:, :128] = W
    for hh in range(H):
        wa[:, 128 + hh] = W[:, hh * C:(hh + 1) * C] @ att_src[hh]
        # one-hot entries are 2.0 (bit shifted to fp16 exponent); halve a_dst
        wa[:, 132 + hh] = 0.5 * (W[:, hh * C:(hh + 1) * C] @ att_dst[hh])
    wa_bf = wa.astype(BF16)

    x_pad = np.zeros((NPAD, P), np.float32)
    x_pad[:N] = x
    x_bf = x_pad.astype(BF16)

    K, groups, idx_maps, bm_maps = _prep_edges(np.asarray(edge_index))
    plan, TOTC = _plan(K, groups)
    bmi_maps = _interleave_bitmaps(plan, TOTC, bm_maps)
    nc = _build_program(K, groups)

    bias_rep = np.tile(bias[None, :], (P, 1)).astype(np.float32)

    in_maps = []
    for c in range(NCORE):
        xc = np.roll(x_bf, -c * B, axis=0)
        in_maps.append({
            "xbf": np.ascontiguousarray(xc),
            "waug": wa_bf,
            "idx_all": idx_maps[c],
            "bm_all": bmi_maps[c],
            "bias_in": bias_rep,
        })
    return nc, in_maps


def kernel(x, edge_index, W, att_src, att_dst, bias):
    nc, in_maps = prepare(x, edge_index, W, att_src, att_dst, bias)
    res = run_bass_kernel_spmd(nc, in_maps, list(range(NCORE)))
    out = np.empty((NPAD, C), np.float32)
    for c in range(NCORE):
        out[c * B:(c + 1) * B] = res.results[c]["out"]
    return out[:N]


# revision 37
# speedup vs baseline: 2.5398x; 1.0001x over previous
"""GATConv forward on 8 Trainium2 NeuronCores (Bass/Tile).

Strategy: destination-node sharding. Host sorts edges by dst, assigns each
core a contiguous dst range (12544 nodes = 98 tiles of 128). Node ids are
cyclically renumbered per core so every core's local nodes are 0..12543 and
the SPMD program is identical across cores; all per-core variation lives in
the input data.

Phase A: per 128-node tile, [h | a_src | 0.5*a_dst] = x @ waug on the PE;
rows packed into a 512B-row HBM table T as [h bf16 (256B) | a_src fp16
(8B) | pad]; 0.5*a_dst for local tiles kept in SBUF (slot-partitioned,
fp16).

Phase B: dst tiles are bin-packed into groups of 3 and processed in a
software pipeline so each engine's in-order queue never head-of-line
blocks on another engine:
  iteration i issues  gather(i)+streams(i+1) | bitexpand(i) | a_dst(i)
  [PE] | z/w/msg(i-1) [DVE+Act] | scatter(i-1) [PE] | epilogue(i-2).
dma_gathers are capped at 8 chunks (1024-descriptor SWDGE ring limit).
One-hot matrices come from host-packed uint16 bitmaps bit-expanded with
(x & (1<<b)) << (14-b), whose u16 result 0x4000 bitcast to fp16 is exactly
2.0 (scale cancels in the softmax ratio; att_dst is pre-halved). a_dst per
edge = tiny PE matmuls s01T^T @ adst; w = exp(lrelu(a_src + a_dst));
messages [w*h | w] scatter into a per-group [128,3,132] PSUM accumulator;
epilogue relu(mean_h num/den + bias).
"""
import sys

sys.path.insert(0, "/opt/trn_rl_repo")
import numpy as np
import ml_dtypes

import concourse.bass as bass
import concourse.mybir as mybir
import concourse.tile as tile
from concourse.bass_utils import run_bass_kernel_spmd
from concourse import bacc

BF16 = ml_dtypes.bfloat16
P = 128
N = 100000
NPAD = 100352          # 784 tiles of 128; 8 cores x 12544
NCORE = 8
B = NPAD // NCORE      # 12544 local nodes per core
TPC = B // P           # 98 tiles per core
NTILE = NPAD // P      # 784 global row tiles
BANK = 32768
NBANK = (NPAD + BANK - 1) // BANK  # 4
NEG = 0.2
H, C = 4, 32
G = 3                  # dst tiles per group (bin-packed)
EW = 256               # T row width in u16 elements (512B)


def _prep_edges(edge_index):
    src0 = edge_index[0].astype(np.int64)
    dst0 = edge_index[1].astype(np.int64)
    loops = np.arange(NPAD, dtype=np.int64)
    src = np.concatenate([src0, loops])
    dst = np.concatenate([dst0, loops])

    per_core = []
    cnts = np.zeros((NCORE, TPC * NBANK), np.int64)
    for c in range(NCORE):
        lo, hi = c * B, (c + 1) * B
        sel = (dst >= lo) & (dst < hi)
        d = dst[sel] - lo
        s = (src[sel] - lo) % NPAD
        t = d >> 7
        sl = d & 127
        bk = s >> 15
        il = s & (BANK - 1)
        q = t * NBANK + bk
        per_core.append((q, il, sl))
        cnts[c] = np.bincount(q, minlength=TPC * NBANK)

    K = np.ceil(cnts.max(axis=0) / P).astype(np.int64).reshape(TPC, NBANK)

    # bin-pack tiles into groups of G, balancing total chunks per group
    w = K.sum(axis=1)
    order = np.argsort(-w, kind="stable")
    ngrp = (TPC + G - 1) // G
    bins = [[] for _ in range(ngrp)]
    load = np.zeros(ngrp, np.int64)
    for t in order:
        cand = [b for b in range(ngrp) if len(bins[b]) < G]
        b = min(cand, key=lambda i: load[i])
        bins[b].append(int(t))
        load[b] += w[t]
    groups = [sorted(b) for b in bins]

    # global chunk layout: (group, bank, tile, chunk)
    qorder = []
    for tiles in groups:
        for bk in range(NBANK):
            for t in tiles:
                qorder.append(t * NBANK + bk)
    qorder = np.array(qorder, np.int64)
    sz_by_q = (K.reshape(-1) * P)
    sz_in_order = sz_by_q[qorder]
    goff_in_order = np.zeros(len(qorder) + 1, np.int64)
    np.cumsum(sz_in_order, out=goff_in_order[1:])
    tot = int(goff_in_order[-1])
    qoff = np.zeros(TPC * NBANK, np.int64)
    qoff[qorder] = goff_in_order[:-1]

    TOTC = tot // P
    idx_maps, bm_maps = [], []
    for c in range(NCORE):
        q, il, sl = per_core[c]
        cnt = cnts[c]
        start = np.zeros(TPC * NBANK + 1, np.int64)
        np.cumsum(cnt, out=start[1:])
        order_e = np.argsort(q, kind="stable")
        qs = q[order_e]
        rank = np.arange(len(qs)) - start[qs]
        pos = qoff[qs] + rank               # global padded edge position
        idx_pad = np.zeros(tot, np.int16)
        idx_pad[pos] = il[order_e].astype(np.int16)

        # idx table: per chunk [16, 8] wrap -> [16, tot/16], replicated to 128
        idx16 = np.ascontiguousarray(
            idx_pad.reshape(TOTC, 8, 16).transpose(2, 0, 1).reshape(16, TOTC * 8))
        idx128 = np.ascontiguousarray(np.tile(idx16, (8, 1)))

        cc = pos >> 7                       # chunk of each real edge
        lane = pos & 127                    # partition lane within chunk
        slv = sl[order_e]                   # slot (dst & 127) of each edge
        # bmg[e, cc*8 + slot//16] bit slot%16  (edge-partitioned, bits=slot)
        bmg = np.zeros((P, TOTC * 8), np.uint16)
        np.bitwise_or.at(bmg, (lane, cc * 8 + (slv >> 4)),
                         (1 << (slv & 15)).astype(np.uint16))
        # bmt[slot, cc*8 + lane//16] bit lane%16 (slot-partitioned, bits=edge)
        bmt = np.zeros((P, TOTC * 8), np.uint16)
        np.bitwise_or.at(bmt, (slv, cc * 8 + (lane >> 4)),
                         (1 << (lane & 15)).astype(np.uint16))
        idx_maps.append(idx128)
        bm_maps.append((bmg, bmt))
    return K, groups, idx_maps, bm_maps


def _plan(K, groups):
    """Per-group program metadata (shared across cores)."""
    plan = []
    cc = 0
    for tiles in groups:
        gathers = []   # (bank, nch, chunk_off_in_group); nch <= 8
        off = 0
        for bk in range(NBANK):
            nch_bk = int(K[tiles, bk].sum())
            for p0 in range(0, nch_bk, 8):
                gathers.append((bk, min(8, nch_bk - p0), off + p0))
            off += nch_bk
        kg = off
        # chunk -> owning tile (local index), in (bank, tile, chunk) order
        owner = []
        for bk in range(NBANK):
            for ti, t in enumerate(tiles):
                owner += [ti] * int(K[t, bk])
        # scatter order: tile-major so accumulation groups don't interleave
        scatter = []   # (t_local, chunk idx, start, stop)
        for ti, t in enumerate(tiles):
            ks = [k for k in range(kg) if owner[k] == ti]
            for i, k in enumerate(ks):
                scatter.append((ti, k, i == 0, i == len(ks) - 1))
        plan.append(dict(tiles=tiles, gathers=gathers, kg=kg, cc0=cc,
                         owner=owner, scatter=scatter))
        cc += kg
    return plan, cc


def _interleave_bitmaps(plan, TOTC, bm_maps):
    out = []
    for bmg, bmt in bm_maps:
        m = np.zeros((P, TOTC * 16), np.uint16)
        for pl in plan:
            c0, kg = pl["cc0"], pl["kg"]
            m[:, c0 * 16:c0 * 16 + kg * 8] = bmg[:, c0 * 8:(c0 + kg) * 8]
            m[:, c0 * 16 + kg * 8:(c0 + kg) * 16] = bmt[:, c0 * 8:(c0 + kg) * 8]
        out.append(m)
    return out


def _build_program(K, groups):
    plan, TOTC = _plan(K, groups)
    NG = len(plan)
    TOT16 = TOTC * 8
    f32, bf16, fp16, fp8, i16, u16 = (
        mybir.dt.float32, mybir.dt.bfloat16, mybir.dt.float16,
        mybir.dt.float8e4, mybir.dt.int16, mybir.dt.uint16)
    AF = mybir.ActivationFunctionType
    OP = mybir.AluOpType

    nc = bacc.Bacc("TRN2", target_bir_lowering=False, debug=False,
                   num_devices=NCORE)
    xbf = nc.dram_tensor("xbf", [NPAD, P], bf16, kind="ExternalInput")
    waug = nc.dram_tensor("waug", [P, 136], bf16, kind="ExternalInput")
    idx_all = nc.dram_tensor("idx_all", [P, TOT16], i16, kind="ExternalInput")
    bm_all = nc.dram_tensor("bm_all", [P, TOTC * 16], u16, kind="ExternalInput")
    bias_in = nc.dram_tensor("bias_in", [P, C], f32, kind="ExternalInput")
    T = nc.dram_tensor("T", [NPAD, EW], u16)
    out_d = nc.dram_tensor("out", [B, C], f32, kind="ExternalOutput")

    Tv = T[:, :].rearrange("(t p) e -> p t e", p=P)         # [128, 784, 128]
    out_v = out_d[:, :].rearrange("(t p) c -> p t c", p=P)  # [128, 98, 32]

    with tile.TileContext(nc) as tc:
        with tc.tile_pool(name="const", bufs=1) as cp:
            waug_sb = cp.tile([P, 136], bf16)
            nc.sync.dma_start(out=waug_sb[:], in_=waug[:, :])
            bias_sb = cp.tile([P, C], f32)
            nc.sync.dma_start(out=bias_sb[:], in_=bias_in[:, :])
            adst_sb = cp.tile([P, TPC, 4], fp16)
            outall_sb = cp.tile([P, TPC, C], f32)

            # ------------- Phase A: T = x @ waug, a_dst table --------------
            # Chunked into few, large DMAs: the tile scheduler serializes
            # any DMA against the next one with a ~5.5us bubble, so one
            # 8192-row transpose-in and one 64-tile T-write-out per chunk
            # bounds that cost to ~13 bubbles total. T writes go through
            # HWDGE (Act queue) -- SWDGE's 1024-descriptor ring cannot take
            # an 8192-descriptor DMA.
            CH = 16384
            NCHUNK = (NPAD + CH - 1) // CH  # 7 (last chunk 2048 rows)
            with tc.tile_pool(name="pax", bufs=2) as pax, \
                 tc.tile_pool(name="pat", bufs=2) as pat, \
                 tc.tile_pool(name="psa", bufs=2, space="PSUM") as psa:
                for ck in range(NCHUNK):
                    r0 = ck * CH
                    nrows = min(CH, NPAD - r0)
                    nt = nrows // P                  # tiles in chunk (<=64)
                    xT = pax.tile([P, CH], bf16, tag="xT")
                    nc.sync.dma_start(out=xT[:, 0:nrows],
                                      in_=xbf[r0:r0 + nrows, :],
                                      transpose=True)
                    Tb = pat.tile([P, CH // P, 132], u16, tag="Tb")
                    for q in range(nt // 4):
                        ps = psa.tile([P, 4, 512], f32, tag="psA",
                                      space="PSUM")
                        for i in range(4):
                            nc.tensor.matmul(
                                out=ps[:, i, 0:136],
                                lhsT=xT[:, (q * 4 + i) * P:(q * 4 + i + 1) * P],
                                rhs=waug_sb[:], start=True, stop=True)
                        q4 = slice(q * 4, (q + 1) * 4)
                        if q % 2 == 0:
                            nc.scalar.activation(
                                out=Tb[:, q4, 0:128].bitcast(bf16),
                                in_=ps[:, :, 0:128], func=AF.Copy)
                        else:
                            nc.vector.tensor_copy(
                                out=Tb[:, q4, 0:128].bitcast(bf16),
                                in_=ps[:, :, 0:128])
                        nc.scalar.activation(
                            out=Tb[:, q4, 128:132].bitcast(fp16),
                            in_=ps[:, :, 128:132], func=AF.Copy)
                        t0 = r0 // P + q * 4
                        if t0 < TPC:
                            nloc = min(4, TPC - t0)
                            nc.scalar.activation(
                                out=adst_sb[:, t0:t0 + nloc, :],
                                in_=ps[:, 0:nloc, 132:136], func=AF.Copy)
                    nc.scalar.dma_start(
                        out=Tv[:, r0 // P:r0 // P + nt, 0:132],
                        in_=Tb[:, 0:nt, :])

            tc.strict_bb_all_engine_barrier()

            # ------------- Phase B: software-pipelined groups --------------
            with tc.tile_pool(name="pgt", bufs=2) as pgt, \
                 tc.tile_pool(name="pix", bufs=2) as pix, \
                 tc.tile_pool(name="pbm", bufs=2) as pbm, \
                 tc.tile_pool(name="ptr", bufs=4) as ptr, \
                 tc.tile_pool(name="pt1", bufs=1) as pt1, \
                 tc.tile_pool(name="psm", bufs=2) as psm, \
                 tc.tile_pool(name="pms", bufs=1) as pms, \
                 tc.tile_pool(name="psb", bufs=4, space="PSUM") as psb, \
                 tc.tile_pool(name="psd", bufs=3, space="PSUM") as psd:
                st = {}  # live per-group tiles
                nregs = {}
                for pl_ in plan:
                    for _, nch_, _ in pl_["gathers"]:
                        nregs.setdefault(nch_ * P, None)
                for v in sorted(nregs):
                    nregs[v] = nc.gpsimd.to_reg(v)

                def issue_streams(g):
                    pl = plan[g]
                    kg, cc0 = pl["kg"], pl["cc0"]
                    idx_t = pix.tile([P, kg * 8], i16, tag="idx")
                    nc.scalar.dma_start(
                        out=idx_t[:], in_=idx_all[:, cc0 * 8:(cc0 + kg) * 8])
                    bm_t = pbm.tile([P, 2, kg * 8], u16, tag="bm")
                    nc.scalar.dma_start(
                        out=bm_t[:].rearrange("p a b -> p (a b)"),
                        in_=bm_all[:, cc0 * 16:(cc0 + kg) * 16])
                    st[g] = {"idx": idx_t, "bm": bm_t}

                def issue_gather(g):
                    pl = plan[g]
                    kg = pl["kg"]
                    gt = pgt.tile([P, kg, EW], u16, tag="gath")
                    idx_t = st[g]["idx"]
                    for bk, nch, off in pl["gathers"]:
                        rows = min(BANK, NPAD - bk * BANK)
                        nc.gpsimd.dma_gather(
                            out_ap=gt[:, off:off + nch, :],
                            in_ap=T[bk * BANK:bk * BANK + rows, :],
                            idxs_ap=idx_t[:, off * 8:(off + nch) * 8],
                            num_idxs=nch * P, num_idxs_reg=nregs[nch * P],
                            elem_size=EW)
                    st[g]["gt"] = gt

                def issue_bitexp(g):
                    pl = plan[g]
                    kg = pl["kg"]
                    k8 = kg * 8
                    bm_t = st[g]["bm"]
                    bxg = ptr.tile([P, k8, 16], u16, tag="bxg")
                    bxt = pt1.tile([P, k8, 16], u16, tag="bxt")
                    for b in range(16):
                        sh = (OP.logical_shift_left if b <= 14
                              else OP.logical_shift_right)
                        nc.vector.tensor_scalar(
                            out=bxt[:, :, b],
                            in0=bm_t[:, 1], scalar1=1 << b, scalar2=abs(14 - b),
                            op0=OP.bitwise_and, op1=sh)
                        nc.vector.tensor_scalar(
                            out=bxg[:, :, b],
                            in0=bm_t[:, 0], scalar1=1 << b, scalar2=abs(14 - b),
                            op0=OP.bitwise_and, op1=sh)
                    st[g]["s01g"] = bxg[:].bitcast(fp16).rearrange(
                        "p (k w) b -> p k (w b)", w=8)
                    st[g]["s01t"] = bxt[:].bitcast(fp16).rearrange(
                        "p (k w) b -> p k (w b)", w=8)

                def issue_adt(g):
                    pl = plan[g]
                    kg, tiles = pl["kg"], pl["tiles"]
                    s01t = st[g]["s01t"]
                    adt_ps = psd.tile([P, kg, 4], f32, tag="adt", space="PSUM")
                    for k in range(kg):
                        nc.tensor.matmul(
                            out=adt_ps[:, k, :], lhsT=s01t[:, k, :],
                            rhs=adst_sb[:, tiles[pl["owner"][k]], :],
                            start=True, stop=True)
                    st[g]["adt"] = adt_ps

                def issue_z(g):
                    pl = plan[g]
                    kg = pl["kg"]
                    gt = st[g]["gt"]
                    aview = gt[:, :, 128:132].bitcast(fp16)  # [P, kg, 4]
                    zt = psm.tile([P, kg, 4], fp16, tag="zt")
                    nc.vector.tensor_tensor(out=zt[:], in0=aview,
                                            in1=st[g]["adt"][:], op=OP.add)
                    lr = psm.tile([P, kg * 4], fp16, tag="lr")
                    nc.vector.scalar_tensor_tensor(
                        out=lr[:], in0=zt[:].rearrange("p k f -> p (k f)"),
                        scalar=NEG, in1=zt[:].rearrange("p k f -> p (k f)"),
                        op0=OP.mult, op1=OP.max)
                    wb = psm.tile([P, kg, 4], fp16, tag="wb")
                    nc.scalar.activation(
                        out=wb[:].rearrange("p k f -> p (k f)"), in_=lr[:],
                        func=AF.Exp)
                    wbx = pms.tile([P, kg, H, C], fp16, tag="wbx")
                    nc.scalar.activation(
                        out=wbx[:],
                        in_=wb[:, :, :, None].to_broadcast([P, kg, H, C]),
                        func=AF.Copy)
                    st[g]["wb"] = wb
                    st[g]["wbx"] = wbx

                def issue_msg2(g):
                    pl = plan[g]
                    kg = pl["kg"]
                    gt = st[g]["gt"]
                    hview = gt[:, :, 0:128].bitcast(bf16)    # [P, kg, 128]
                    msg = pms.tile([P, kg, 132], fp16, tag="msg")
                    nc.vector.tensor_tensor(
                        out=msg[:, :, 0:128],
                        in0=hview,
                        in1=st[g]["wbx"][:].rearrange("p k h c -> p (k h c)")
                            .rearrange("p (k f) -> p k f", k=kg),
                        op=OP.mult)
                    nc.scalar.activation(out=msg[:, :, 128:132],
                                         in_=st[g]["wb"][:], func=AF.Copy)
                    st[g]["msg"] = msg

                def issue_scatter(g):
                    pl = plan[g]
                    msg, s01g = st[g]["msg"], st[g]["s01g"]
                    acc = psb.tile([P, G, 132], f32, tag="acc", space="PSUM")
                    for ti, k, first, last in pl["scatter"]:
                        nc.tensor.matmul(out=acc[:, ti, :], lhsT=s01g[:, k, :],
                                         rhs=msg[:, k, 0:132],
                                         start=first, stop=last)
                    st[g]["acc"] = acc

                def issue_epilogue(g):
                    pl = plan[g]
                    tiles = pl["tiles"]
                    ng = len(tiles)
                    acc = st[g]["acc"]
                    den = psm.tile([P, G, 4], f32, tag="den")
                    nc.vector.reciprocal(out=den[:, 0:ng, :],
                                         in_=acc[:, 0:ng, 128:132])
                    tmp = psm.tile([P, G, P], f32, tag="tmp")
                    nc.vector.tensor_tensor(
                        out=tmp[:, 0:ng, :].rearrange("p g (h c) -> p g h c", h=H),
                        in0=acc[:, 0:ng, 0:128].rearrange("p g (h c) -> p g h c", h=H),
                        in1=den[:, 0:ng, :, None].to_broadcast([P, ng, H, C]),
                        op=OP.mult)
                    hsum = psm.tile([P, G, C], f32, tag="hsum")
                    nc.vector.tensor_reduce(
                        out=hsum[:, 0:ng, :],
                        in_=tmp[:, 0:ng, :].rearrange("p g (h c) -> p g c h", h=H),
                        axis=mybir.AxisListType.X, op=OP.add)
                    badd = psm.tile([P, G, C], f32, tag="badd")
                    nc.vector.scalar_tensor_tensor(
                        out=badd[:, 0:ng, :], in0=hsum[:, 0:ng, :],
                        scalar=1.0 / H,
                        in1=bias_sb[:, None, :].to_broadcast([P, ng, C]),
                        op0=OP.mult, op1=OP.add)
                    for ti, t in enumerate(tiles):
                        nc.vector.tensor_scalar_max(
                            out=outall_sb[:, t, :], in0=badd[:, ti, :],
                            scalar1=0.0)
                    del st[g]

                issue_streams(0)
                for i in range(NG + 2):
                    if 1 <= i <= NG:
                        issue_z(i - 1)
                    if i + 1 < NG:
                        issue_streams(i + 1)
                    if i < NG:
                        issue_gather(i)
                        issue_bitexp(i)
                        issue_adt(i)
                    if 1 <= i <= NG:
                        issue_msg2(i - 1)
                        issue_scatter(i - 1)
                    if i >= 2:
                        issue_epilogue(i - 2)
                nc.sync.dma_start(out=out_v[:, :, :], in_=outall_sb[:])
    nc.compile()
    return nc


def prepare(x, edge_index, W, att_src, att_dst, bias):
    x = np.asarray(x, np.float32)
    W = np.asarray(W, np.float32)
    att_src = np.asarray(att_src, np.float32)
    att_dst = np.asarray(att_dst, np.float32)
    bias = np.asarray(bias, np.float32)

    wa = np.zeros((P, 136), np.float32)
    wa[:, :128] = W
    for hh in range(H):
        wa[:, 128 + hh] = W[:, hh * C:(hh + 1) * C] @ att_src[hh]
        # one-hot entries are 2.0 (bit shifted to fp16 exponent); halve a_dst
        wa[:, 132 + hh] = 0.5 * (W[:, hh * C:(hh + 1) * C] @ att_dst[hh])
    wa_bf = wa.astype(BF16)

    x_pad = np.zeros((NPAD, P), np.float32)
    x_pad[:N] = x
    x_bf = x_pad.astype(BF16)

    K, groups, idx_maps, bm_maps = _prep_edges(np.asarray(edge_index))
    plan, TOTC = _plan(K, groups)
    bmi_maps = _interleave_bitmaps(plan, TOTC, bm_maps)
    nc = _build_program(K, groups)

    bias_rep = np.tile(bias[None, :], (P, 1)).astype(np.float32)

    in_maps = []
    for c in range(NCORE):
        xc = np.roll(x_bf, -c * B, axis=0)
        in_maps.append({
            "xbf": np.ascontiguousarray(xc),
            "waug": wa_bf,
            "idx_all": idx_maps[c],
            "bm_all": bmi_maps[c],
            "bias_in": bias_rep,
        })
    return nc, in_maps


def kernel(x, edge_index, W, att_src, att_dst, bias):
    nc, in_maps = prepare(x, edge_index, W, att_src, att_dst, bias)
    res = run_bass_kernel_spmd(nc, in_maps, list(range(NCORE)))
    out = np.empty((NPAD, C), np.float32)
    for c in range(NCORE):
        out[c * B:(c + 1) * B] = res.results[c]["out"]
    return out[:N]
